# revision 120
# baseline (speedup 1.0000x reference)
"""CrossAttention Trainium2 kernel (bf16 pipeline).

Problem: nn_CrossAttention (B=4, N=M=1024, DIM=CTX_DIM=1024, H=16, DH=64).

Sharding: 8 cores = batch (4) x head-group (2 groups of 8 heads).
Each core computes, for its (b, g):
    q = rope(x[b] @ Wq[:, g])
    k = rope(context[b] @ Wk[:, g]);  v = context[b] @ Wv[:, g]
    attn = softmax(q k^T / sqrt(dh))     (mask is all-ones by construction)
    partial_out[b,g] = (attn @ v) @ Wout[g, :]
Host transposes x/context per batch and casts everything to bf16; it sums the
two head-group partials per batch and adds bout.

All matmuls run in bf16 (fp32 PSUM accumulation).  bf16 moving data streams at
1 cycle/row and the separate Ldweights instructions keep the PE p-state ramp
warm.  Dots/projection PSUM tiles are [128, 1024] (2 banks) so the Activation
engine amortizes its access latency over 1024-wide exp/cast chunks.

Engine assignment:
    PE    : all matmuls (projections, dots, attn@v, final)
    Act   : psum->bf16 casts feeding rope, exp(dots) -> es bf16, half the
            final copies
    DVE   : rope muls (bf16 SBUF, 2x perf mode), denominator reciprocal
            (reads PSUM rows 64-127 directly), normalize-mult fused with the
            psum->sbuf move of attn@v outputs, half the final copies
    Pool  : wq/cos/sin/wk loads via software DGE (fastest first-chunk
            latency; engine otherwise idle)
    SP    : x/context/wv/wo loads, half the rope-rotation DMAs, stores

Key tricks:
  - rope's rotate_half is a partition permutation (p XOR 32): done by small
    SBUF->SBUF DMAs (2 on SP, 2 on Act per chunk), because DVE tensor-tensor
    ops require equal SBUF start partitions (walrus
    checkSBSameStartPartition) while DMA addresses partitions freely.
  - v carries 64 ones-COLUMNS, so the attn@v matmul replicates the softmax
    denominator across PSUM rows 64-127 for free (matmul cost is moving-rows
    only); normalization is then reciprocal + one mul per half, all
    same-start.
  - head 0's dots+exp are interleaved with the v projection so the Act
    engine is warm when the (Act-exp-bound) attention loop starts; in the
    loop, attn_v(h) chunks are issued before the exp-ring-gated dots(h+1)
    chunks so the in-order PE queue never head-blocks.
  - the kc=0,1 half of the final projection for output chunks 0-3 runs
    inside attention iters 4-5 (psV ring slack, exp-paced PE slack); phase C
    then alternates folded chunks (kc 2,3 + one wide DVE add of the partial)
    with full chunks so the adds hide behind matmul time.
  - gpsimd partition_broadcast and rearrange-split-partition DMA APs both
    break on real hardware despite passing CoreSim/TimelineSim -- avoided.
"""

import os
import numpy as np

B, N, M = 4, 1024, 1024
DIM = 1024
H, DH = 16, 64
ISH = 512  # inner shard per core (8 heads * 64)
SCALE = DH ** -0.5
P = 128

_CACHE = {}
_LAST_EXEC_NS = None


def _build_program():
    from contextlib import ExitStack

    import concourse.tile as tile
    from concourse import bacc, mybir

    f32 = mybir.dt.float32
    bf16 = mybir.dt.bfloat16
    Exp = mybir.ActivationFunctionType.Exp
    Copy = mybir.ActivationFunctionType.Copy

    nc = bacc.Bacc("TRN2", target_bir_lowering=False, debug=False, num_devices=8)

    xbT = nc.dram_tensor("xbT", [DIM, N], bf16, kind="ExternalInput").ap()
    cxT = nc.dram_tensor("cxT", [DIM, M], bf16, kind="ExternalInput").ap()
    wq = nc.dram_tensor("wq", [DIM, ISH], bf16, kind="ExternalInput").ap()
    wk = nc.dram_tensor("wk", [DIM, ISH], bf16, kind="ExternalInput").ap()
    wv = nc.dram_tensor("wv", [DIM, ISH], bf16, kind="ExternalInput").ap()
    wo = nc.dram_tensor("wo", [ISH, DIM], bf16, kind="ExternalInput").ap()
    cos2 = nc.dram_tensor("cos2", [P, N], bf16, kind="ExternalInput").ap()
    sin2 = nc.dram_tensor("sin2", [P, N], bf16, kind="ExternalInput").ap()
    out = nc.dram_tensor("out", [N, DIM], bf16, kind="ExternalOutput").ap()

    with tile.TileContext(nc) as tc, ExitStack() as ctx:
        const = ctx.enter_context(tc.tile_pool(name="const", bufs=1))
        wpool = ctx.enter_context(tc.tile_pool(name="wpool", bufs=2))
        qk = ctx.enter_context(tc.tile_pool(name="qk", bufs=1))
        vpool = ctx.enter_context(tc.tile_pool(name="vpool", bufs=8))
        tmpp = ctx.enter_context(tc.tile_pool(name="tmpp", bufs=8))

        wq_sb = wpool.tile([P, 8, ISH], bf16, tag="w")
        wk_sb = wpool.tile([P, 8, ISH], bf16, tag="w")
        wv_sb = wpool.tile([P, 8, ISH], bf16, tag="w")
        for k in range(8):
            nc.gpsimd.dma_start(wq_sb[:, k, :], wq[k * P:(k + 1) * P, :])
        cos_sb = const.tile([P, N], bf16, tag="cos")
        nc.gpsimd.dma_start(cos_sb[:], cos2)
        sin_sb = const.tile([P, N], bf16, tag="sin")
        nc.gpsimd.dma_start(sin_sb[:], sin2)
        for k in range(8):
            nc.gpsimd.dma_start(wk_sb[:, k, :], wk[k * P:(k + 1) * P, :])

        # ---- phase A: projections (xT/ctxT big tiles live only here)
        psAB = ctx.enter_context(ExitStack())
        psD = psAB.enter_context(tc.tile_pool(name="psD", bufs=2, space="PSUM"))
        psV = psAB.enter_context(tc.tile_pool(name="psV", bufs=4, space="PSUM"))
        epool = ctx.enter_context(tc.tile_pool(name="epool", bufs=16))
        with tc.tile_pool(name="bigT", bufs=2) as bigT:

            def project_rope(xT, w_sb, tag):
                dst = qk.tile([P, 4, N], bf16, tag=tag)
                for ic in range(4):
                    ps = psD.tile([P, N], f32, tag="mm")
                    for k in range(8):
                        for ns in range(2):
                            lt = w_sb[:, k, ic * P:(ic + 1) * P]
                            if w_sb is wq_sb and ic == 0 and k == 0:
                                lt = wq00[:]
                            nc.tensor.matmul(
                                ps[:, ns * 512:(ns + 1) * 512],
                                lhsT=lt,
                                rhs=xT[:, k, ns * 512:(ns + 1) * 512],
                                start=(k == 0),
                                stop=(k == 7),
                            )
                    qc = tmpp.tile([P, N], bf16, tag="qc")
                    nc.scalar.activation(qc[:], ps[:], Copy)
                    # rope: dst = qc * cos + rotate_half(qc) * sin_signed.
                    # The partition rotation (p -> p XOR 32) runs on the DMA
                    # engines: DVE tensor-tensor ops require equal SBUF start
                    # partitions (walrus checkSBSameStartPartition), and DMA
                    # addresses partitions freely.  Issue split across the SP
                    # and DVE queues to fit their sequencer budgets.
                    qcr = tmpp.tile([P, N], bf16, tag="qcr")
                    for blk in range(4):
                        d0 = blk * 32
                        s0 = (blk ^ 1) * 32
                        eng = nc.sync if blk % 2 == 0 else nc.scalar
                        eng.dma_start(
                            qcr[d0:d0 + 32, :], qc[s0:s0 + 32, :]
                        )
                    dsl = dst[:, ic, :]
                    nc.vector.tensor_mul(out=dsl, in0=qc[:], in1=cos_sb[:])
                    tmp = tmpp.tile([P, N], bf16, tag="tmp")
                    nc.vector.tensor_mul(out=tmp[:], in0=qcr[:], in1=sin_sb[:])
                    nc.vector.tensor_add(out=dsl, in0=dsl, in1=tmp[:])
                return dst

            # DMA issue plan: SP carries wq0 (fastest path for the first
            # matmul) then x/context/wv; Act queue carries wq1-7 in parallel
            # and is free for the rope casts by ~5us; Pool carries cos/sin/wk
            # (software DGE, idle engine).
            wq00 = bigT.tile([P, P], bf16, tag="wq00")
            nc.scalar.dma_start(wq00[:], wq[0:P, 0:P])
            xT = bigT.tile([P, 8, N], bf16, tag="bigT")
            for k in range(8):
                nc.sync.dma_start(xT[:, k, :], xbT[k * P:(k + 1) * P, :])
            cT = bigT.tile([P, 8, N], bf16, tag="bigT")
            for k in range(8):
                nc.sync.dma_start(cT[:, k, :], cxT[k * P:(k + 1) * P, :])
            for k in range(8):
                nc.sync.dma_start(wv_sb[:, k, :], wv[k * P:(k + 1) * P, :])
            qT = project_rope(xT, wq_sb, "qT")
            kT = project_rope(cT, wk_sb, "kT")

            def dots_exp0_mch(mch, es):
                # head 0's dots+exp through the (phase-A-idle) attention psum
                # ring, interleaved with the v projection so the Act engine
                # stays busy through phase A's tail
                e = epool.tile([P, N], bf16, tag="e")
                for ns in range(2):
                    psd = psV.tile([P, 512], f32, tag="av")
                    nc.tensor.matmul(
                        psd[:],
                        lhsT=kT[0:64, 0, mch * P:(mch + 1) * P],
                        rhs=qT[0:64, 0, ns * 512:(ns + 1) * 512],
                        start=True,
                        stop=True,
                    )
                    nc.scalar.activation(
                        e[:, ns * 512:(ns + 1) * 512], psd[:], Exp,
                        scale=SCALE,
                    )
                es.append(e)

            vsb = []
            es0 = []
            for mp in range(4):
                ps = psD.tile([P, N], f32, tag="mm")
                for half in range(2):
                    mch = mp * 2 + half
                    for k in range(8):
                        nc.tensor.matmul(
                            ps[:, half * 512:(half + 1) * 512],
                            lhsT=cT[:, k, mch * P:(mch + 1) * P],
                            rhs=wv_sb[:, k, :],
                            start=(k == 0),
                            stop=(k == 7),
                        )
                for half in range(2):
                    # 64 ones-columns: the attn@v matmul replicates the
                    # softmax denominator across PSUM rows 64-127, so the
                    # partition broadcast of 1/denom costs nothing
                    vt = vpool.tile([P, 8, 2 * DH], bf16, tag="v")
                    nc.scalar.activation(
                        vt[:, :, 0:DH],
                        ps[:, half * 512:(half + 1) * 512].rearrange(
                            "p (h d) -> p h d", d=DH
                        ),
                        Copy,
                    )
                    nc.vector.memset(vt[:, :, DH:2 * DH], 1.0)
                    vsb.append(vt)
                dots_exp0_mch(2 * mp, es0)
                dots_exp0_mch(2 * mp + 1, es0)

        # ---- phase B: attention (bigT space now free)
        rcp = ctx.enter_context(tc.tile_pool(name="rcp", bufs=4))
        rbp = ctx.enter_context(tc.tile_pool(name="rbp", bufs=4))
        drp = ctx.enter_context(tc.tile_pool(name="drp", bufs=4, space="DRAM"))
        opool = ctx.enter_context(tc.tile_pool(name="opool", bufs=6))

        aoT = qk.tile([P, 4, N], bf16, tag="aoT")

        wo_sb = wpool.tile([P, 4, DIM], bf16, tag="w")
        for k in range(4):
            nc.sync.dma_start(wo_sb[:, k, :], wo[k * P:(k + 1) * P, :])

        def denom_normalize(h, pos):
            # PSUM rows 64-127 already hold the denominator replicated (ones
            # columns of v): move to sbuf, reciprocal, normalize.  All SBUF
            # operand pairs share start partitions.
            t2, r0 = h // 2, (h % 2) * 64
            rb = rbp.tile([P, N], f32, tag="rb")
            for ns in range(2):
                nsl = slice(ns * 512, (ns + 1) * 512)
                with nc.allow_low_precision(reason="softmax denom recip"):
                    nc.vector.reciprocal(
                        out=rb[r0:r0 + 64, nsl], in_=pos[ns][DH:2 * DH, :]
                    )
                nc.vector.tensor_mul(
                    out=aoT[r0:r0 + 64, t2, nsl],
                    in0=pos[ns][0:DH, :],
                    in1=rb[r0:r0 + 64, nsl],
                )

        # Main attention loop.  dots(h+1) and attn_v(h) are interleaved at
        # chunk granularity: the dots matmuls are gated by the exp-paced psD
        # ring, and the in-order PE queue would otherwise head-block the
        # (dependency-free) attn_v matmuls behind them.
        o1pool = ctx.enter_context(tc.tile_pool(name="o1pool", bufs=4))
        o1 = []
        es_cur = es0
        for h in range(8):
            if h < 7:
                t2, r0 = (h + 1) // 2, ((h + 1) % 2) * 64
                qh = qT[r0:r0 + 64, t2, :]
                kh = kT[r0:r0 + 64, t2, :]
            es_next = []
            pos = [psV.tile([P, 512], f32, tag="av", name=f"po{_i}")
                   for _i in range(2)]
            for mch in range(8):
                for ns in range(2):
                    nc.tensor.matmul(
                        pos[ns][:],
                        lhsT=vsb[mch][:, h, :],
                        rhs=es_cur[mch][:, ns * 512:(ns + 1) * 512],
                        start=(mch == 0),
                        stop=(mch == 7),
                    )
                if h < 7:
                    psd = psD.tile([P, N], f32, tag="mm")
                    for ns in range(2):
                        nc.tensor.matmul(
                            psd[:, ns * 512:(ns + 1) * 512],
                            lhsT=kh[:, mch * P:(mch + 1) * P],
                            rhs=qh[:, ns * 512:(ns + 1) * 512],
                            start=True,
                            stop=True,
                        )
                    e = epool.tile([P, N], bf16, tag="e")
                    nc.scalar.activation(e[:], psd[:], Exp, scale=SCALE)
                    es_next.append(e)
            denom_normalize(h, pos)
            es_cur = es_next
            # fold the kc=0,1 half of the final projection for nch 0-3 into
            # iters 4-5: psV has two spare ring slots there and the iters are
            # exp-paced with ~1.5us of PE slack
            if h in (4, 5):
                for nch in (2 * (h - 4), 2 * (h - 4) + 1):
                    o1t = o1pool.tile([P, N], f32, tag="o1",
                                      name=f"o1_{nch}")
                    for cc in range(2):
                        pf1 = psV.tile([P, 512], f32, tag="av")
                        for kc in range(2):
                            nc.tensor.matmul(
                                pf1[:],
                                lhsT=aoT[:, kc, nch * P:(nch + 1) * P],
                                rhs=wo_sb[:, kc, cc * 512:(cc + 1) * 512],
                                start=(kc == 0),
                                stop=(kc == 1),
                            )
                        nc.vector.tensor_copy(
                            out=o1t[:, cc * 512:(cc + 1) * 512], in_=pf1[:]
                        )
                    o1.append(o1t)

        # ---- final projection.  psD ring is free immediately (unlike psV,
        # whose last slots wait on norm(7)); only the kc=3 matmuls depend on
        # the last head's normalize chain.  Folded chunks (kc 2,3 + add of
        # the phase-B partial) alternate with full chunks so the DVE adds
        # hide behind the full chunks' matmul time.
        # Tiny Copy first: absorbs the Exp->Copy activation-table reload
        # while the PE is still on the first output chunk.
        warm = opool.tile([P, 8], f32, tag="warm")
        nc.scalar.activation(warm[:], cos_sb[:, 0:8], Copy)
        for nch in (0, 4, 1, 5, 2, 6, 3, 7):
            folded = nch < 4
            ot = opool.tile([P, N], bf16, tag="o")
            if folded:
                # folded chunks run through the psV ring (free in phase C):
                # no contention with the unfolded chunks' psD ring
                for cc in range(2):
                    ql = slice(cc * 512, (cc + 1) * 512)
                    pfh = psV.tile([P, 512], f32, tag="av")
                    for kc in range(2, 4):
                        nc.tensor.matmul(
                            pfh[:],
                            lhsT=aoT[:, kc, nch * P:(nch + 1) * P],
                            rhs=wo_sb[:, kc, cc * 512:(cc + 1) * 512],
                            start=(kc == 2),
                            stop=(kc == 3),
                        )
                    nc.vector.tensor_add(
                        out=ot[:, ql], in0=pfh[:], in1=o1[nch][:, ql]
                    )
                nc.sync.dma_start(out[nch * P:(nch + 1) * P, :], ot[:])
            else:
                # last chunk only: cc-outer so each 512-half's accumulation
                # group closes after its own 4 matmuls and the first
                # half-copy starts ~0.85us before the chunk's last matmul
                # (for earlier chunks kc-outer defers the norm(7)-gated kc=3)
                pf = psD.tile([P, N], f32, tag="mm")
                loops = ([(cc, kc) for cc in range(2) for kc in range(4)]
                         if nch == 7 else
                         [(cc, kc) for kc in range(4) for cc in range(2)])
                for cc, kc in loops:
                    nc.tensor.matmul(
                        pf[:, cc * 512:(cc + 1) * 512],
                        lhsT=aoT[:, kc, nch * P:(nch + 1) * P],
                        rhs=wo_sb[:, kc, cc * 512:(cc + 1) * 512],
                        start=(kc == 0),
                        stop=(kc == 3),
                    )
                if nch == 7:
                    # last chunk: per-half copies on the (now idle) Act
                    # engine with per-half stores — lowest drain latency
                    for q in range(2):
                        ql = slice(q * 512, (q + 1) * 512)
                        nc.scalar.activation(ot[:, ql], pf[:, ql], Copy)
                        nc.sync.dma_start(
                            out[nch * P:(nch + 1) * P, ql], ot[:, ql]
                        )
                else:
                    nc.scalar.activation(ot[:, 0:512], pf[:, 0:512], Copy)
                    nc.vector.tensor_copy(
                        out=ot[:, 512:1024], in_=pf[:, 512:1024]
                    )
                    nc.sync.dma_start(out[nch * P:(nch + 1) * P, :], ot[:])

    nc.compile()
    return nc


def _get_program():
    if "nc" not in _CACHE:
        _CACHE["nc"] = _build_program()
    return _CACHE["nc"]


def make_in_maps(x, context, rotary_pos, Wq, Wkv, Wout):
    from ml_dtypes import bfloat16

    x = np.asarray(x, dtype=np.float32)
    context = np.asarray(context, dtype=np.float32)
    rotary_pos = np.asarray(rotary_pos, dtype=np.float32)
    Wq = np.asarray(Wq, dtype=np.float32)
    Wkv = np.asarray(Wkv, dtype=np.float32)
    Wout = np.asarray(Wout, dtype=np.float32)

    def b16(a):
        return np.ascontiguousarray(a).astype(bfloat16)

    cosT = np.cos(rotary_pos).T  # [64, 1024]
    sinT = np.sin(rotary_pos).T
    sin_signed = np.concatenate([-sinT[:32], sinT[32:]], axis=0)
    cos2 = b16(np.vstack([cosT, cosT]))
    sin2 = b16(np.vstack([sin_signed, sin_signed]))

    in_maps = []
    for core in range(8):
        b, g = core // 2, core % 2
        cs = slice(g * ISH, (g + 1) * ISH)
        in_maps.append({
            "xbT": b16(x[b].T),
            "cxT": b16(context[b].T),
            "wq": b16(Wq[:, cs]),
            "wk": b16(Wkv[:, g * ISH:(g + 1) * ISH]),
            "wv": b16(Wkv[:, H * DH + g * ISH:H * DH + (g + 1) * ISH]),
            "wo": b16(Wout[cs, :]),
            "cos2": cos2,
            "sin2": sin2,
        })
    return in_maps


def kernel(x, context, mask, context_mask, rotary_pos, Wq, Wkv, Wout, bout):
    global _LAST_EXEC_NS
    from concourse.bass_utils import run_bass_kernel_spmd

    nc = _get_program()
    in_maps = make_in_maps(x, context, rotary_pos, Wq, Wkv, Wout)

    trace = bool(os.environ.get("BASS_KERNEL_TRACE"))
    res = run_bass_kernel_spmd(nc, in_maps, core_ids=list(range(8)), trace=trace)
    _LAST_EXEC_NS = res.exec_time_ns
    _CACHE["last_results"] = res

    bout = np.asarray(bout, dtype=np.float32)
    full = np.empty((B, N, DIM), dtype=np.float32)
    for b in range(B):
        full[b] = (
            res.results[2 * b]["out"].astype(np.float32)
            + res.results[2 * b + 1]["out"].astype(np.float32)
            + bout
        )
    return full


# revision 123
# speedup vs baseline: 1.0109x; 1.0109x over previous
"""CrossAttention Trainium2 kernel (bf16 pipeline).

Problem: nn_CrossAttention (B=4, N=M=1024, DIM=CTX_DIM=1024, H=16, DH=64).

Sharding: 8 cores = batch (4) x head-group (2 groups of 8 heads).
Each core computes, for its (b, g):
    q = rope(x[b] @ Wq[:, g])
    k = rope(context[b] @ Wk[:, g]);  v = context[b] @ Wv[:, g]
    attn = softmax(q k^T / sqrt(dh))     (mask is all-ones by construction)
    partial_out[b,g] = (attn @ v) @ Wout[g, :]
Host transposes x/context per batch and casts everything to bf16; it sums the
two head-group partials per batch and adds bout.

All matmuls run in bf16 (fp32 PSUM accumulation).  bf16 moving data streams at
1 cycle/row and the separate Ldweights instructions keep the PE p-state ramp
warm.  Dots/projection PSUM tiles are [128, 1024] (2 banks) so the Activation
engine amortizes its access latency over 1024-wide exp/cast chunks.

Engine assignment:
    PE    : all matmuls (projections, dots, attn@v, final)
    Act   : psum->bf16 casts feeding rope, exp(dots) -> es bf16, half the
            final copies
    DVE   : rope muls (bf16 SBUF, 2x perf mode), denominator reciprocal
            (reads PSUM rows 64-127 directly), normalize-mult fused with the
            psum->sbuf move of attn@v outputs, half the final copies
    Pool  : wq/cos/sin/wk loads via software DGE (fastest first-chunk
            latency; engine otherwise idle)
    SP    : x/context/wv/wo loads, half the rope-rotation DMAs, stores

Key tricks:
  - rope's rotate_half is a partition permutation (p XOR 32): done by small
    SBUF->SBUF DMAs (2 on SP, 2 on Act per chunk), because DVE tensor-tensor
    ops require equal SBUF start partitions (walrus
    checkSBSameStartPartition) while DMA addresses partitions freely.
  - v carries 64 ones-COLUMNS, so the attn@v matmul replicates the softmax
    denominator across PSUM rows 64-127 for free (matmul cost is moving-rows
    only); normalization is then reciprocal + one mul per half, all
    same-start.
  - head 0's dots+exp are interleaved with the v projection so the Act
    engine is warm when the (Act-exp-bound) attention loop starts; in the
    loop, attn_v(h) chunks are issued before the exp-ring-gated dots(h+1)
    chunks so the in-order PE queue never head-blocks.
  - the kc=0,1 half of the final projection for output chunks 0-3 runs
    inside attention iters 4-5 (psV ring slack, exp-paced PE slack); phase C
    then alternates folded chunks (kc 2,3 + one wide DVE add of the partial)
    with full chunks so the adds hide behind matmul time.
  - gpsimd partition_broadcast and rearrange-split-partition DMA APs both
    break on real hardware despite passing CoreSim/TimelineSim -- avoided.
"""

import os
import numpy as np

B, N, M = 4, 1024, 1024
DIM = 1024
H, DH = 16, 64
ISH = 512  # inner shard per core (8 heads * 64)
SCALE = DH ** -0.5
P = 128

_CACHE = {}
_LAST_EXEC_NS = None


def _build_program():
    from contextlib import ExitStack

    import concourse.tile as tile
    from concourse import bacc, mybir

    f32 = mybir.dt.float32
    bf16 = mybir.dt.bfloat16
    Exp = mybir.ActivationFunctionType.Exp
    Copy = mybir.ActivationFunctionType.Copy

    nc = bacc.Bacc("TRN2", target_bir_lowering=False, debug=False, num_devices=8)

    xbT = nc.dram_tensor("xbT", [DIM, N], bf16, kind="ExternalInput").ap()
    cxT = nc.dram_tensor("cxT", [DIM, M], bf16, kind="ExternalInput").ap()
    wq = nc.dram_tensor("wq", [DIM, ISH], bf16, kind="ExternalInput").ap()
    wk = nc.dram_tensor("wk", [DIM, ISH], bf16, kind="ExternalInput").ap()
    wv = nc.dram_tensor("wv", [DIM, ISH], bf16, kind="ExternalInput").ap()
    wo = nc.dram_tensor("wo", [ISH, DIM], bf16, kind="ExternalInput").ap()
    cos2 = nc.dram_tensor("cos2", [P, N], bf16, kind="ExternalInput").ap()
    sin2 = nc.dram_tensor("sin2", [P, N], bf16, kind="ExternalInput").ap()
    out = nc.dram_tensor("out", [N, DIM], bf16, kind="ExternalOutput").ap()

    with tile.TileContext(nc) as tc, ExitStack() as ctx:
        const = ctx.enter_context(tc.tile_pool(name="const", bufs=1))
        wpool = ctx.enter_context(tc.tile_pool(name="wpool", bufs=2))
        qk = ctx.enter_context(tc.tile_pool(name="qk", bufs=1))
        vpool = ctx.enter_context(tc.tile_pool(name="vpool", bufs=8))
        tmpp = ctx.enter_context(tc.tile_pool(name="tmpp", bufs=8))

        wq_sb = wpool.tile([P, 8, ISH], bf16, tag="w")
        wk_sb = wpool.tile([P, 8, ISH], bf16, tag="w")
        wv_sb = wpool.tile([P, 8, ISH], bf16, tag="w")
        for k in range(8):
            nc.gpsimd.dma_start(wq_sb[:, k, :], wq[k * P:(k + 1) * P, :])
        cos_sb = const.tile([P, N], bf16, tag="cos")
        nc.gpsimd.dma_start(cos_sb[:], cos2)
        sin_sb = const.tile([P, N], bf16, tag="sin")
        nc.gpsimd.dma_start(sin_sb[:], sin2)
        for k in range(8):
            nc.gpsimd.dma_start(wk_sb[:, k, :], wk[k * P:(k + 1) * P, :])

        # ---- phase A: projections (xT/ctxT big tiles live only here)
        psAB = ctx.enter_context(ExitStack())
        psD = psAB.enter_context(tc.tile_pool(name="psD", bufs=2, space="PSUM"))
        psV = psAB.enter_context(tc.tile_pool(name="psV", bufs=4, space="PSUM"))
        epool = ctx.enter_context(tc.tile_pool(name="epool", bufs=16))
        with tc.tile_pool(name="bigT", bufs=2) as bigT:

            def project_rope(xT, w_sb, tag):
                dst = qk.tile([P, 4, N], bf16, tag=tag)
                for ic in range(4):
                    ps = psD.tile([P, N], f32, tag="mm")
                    for k in range(8):
                        for ns in range(2):
                            lt = w_sb[:, k, ic * P:(ic + 1) * P]
                            if w_sb is wq_sb and ic == 0 and k == 0:
                                lt = wq00[:]
                            nc.tensor.matmul(
                                ps[:, ns * 512:(ns + 1) * 512],
                                lhsT=lt,
                                rhs=xT[:, k, ns * 512:(ns + 1) * 512],
                                start=(k == 0),
                                stop=(k == 7),
                            )
                    qc = tmpp.tile([P, N], bf16, tag="qc")
                    nc.scalar.activation(qc[:], ps[:], Copy)
                    # rope: dst = qc * cos + rotate_half(qc) * sin_signed.
                    # The partition rotation (p -> p XOR 32) runs on the DMA
                    # engines: DVE tensor-tensor ops require equal SBUF start
                    # partitions (walrus checkSBSameStartPartition), and DMA
                    # addresses partitions freely.  Issue split across the SP
                    # and DVE queues to fit their sequencer budgets.
                    qcr = tmpp.tile([P, N], bf16, tag="qcr")
                    for blk in range(4):
                        d0 = blk * 32
                        s0 = (blk ^ 1) * 32
                        eng = nc.sync if blk % 2 == 0 else nc.scalar
                        eng.dma_start(
                            qcr[d0:d0 + 32, :], qc[s0:s0 + 32, :]
                        )
                    dsl = dst[:, ic, :]
                    nc.vector.tensor_mul(out=dsl, in0=qc[:], in1=cos_sb[:])
                    tmp = tmpp.tile([P, N], bf16, tag="tmp")
                    nc.vector.tensor_mul(out=tmp[:], in0=qcr[:], in1=sin_sb[:])
                    nc.vector.tensor_add(out=dsl, in0=dsl, in1=tmp[:])
                return dst

            # DMA issue plan: SP carries wq0 (fastest path for the first
            # matmul) then x/context/wv; Act queue carries wq1-7 in parallel
            # and is free for the rope casts by ~5us; Pool carries cos/sin/wk
            # (software DGE, idle engine).
            wq00 = bigT.tile([P, P], bf16, tag="wq00")
            nc.scalar.dma_start(wq00[:], wq[0:P, 0:P])
            xT = bigT.tile([P, 8, N], bf16, tag="bigT")
            for k in range(8):
                nc.sync.dma_start(xT[:, k, :], xbT[k * P:(k + 1) * P, :])
            cT = bigT.tile([P, 8, N], bf16, tag="bigT")
            for k in range(8):
                nc.sync.dma_start(cT[:, k, :], cxT[k * P:(k + 1) * P, :])
            for k in range(8):
                nc.sync.dma_start(wv_sb[:, k, :], wv[k * P:(k + 1) * P, :])
            qT = project_rope(xT, wq_sb, "qT")
            kT = project_rope(cT, wk_sb, "kT")

            def dots_exp0_mch(mch, es):
                # head 0's dots+exp through the (phase-A-idle) attention psum
                # ring, interleaved with the v projection so the Act engine
                # stays busy through phase A's tail
                e = epool.tile([P, N], bf16, tag="e")
                for ns in range(2):
                    psd = psV.tile([P, 512], f32, tag="av")
                    nc.tensor.matmul(
                        psd[:],
                        lhsT=kT[0:64, 0, mch * P:(mch + 1) * P],
                        rhs=qT[0:64, 0, ns * 512:(ns + 1) * 512],
                        start=True,
                        stop=True,
                    )
                    nc.scalar.activation(
                        e[:, ns * 512:(ns + 1) * 512], psd[:], Exp,
                        scale=SCALE,
                    )
                es.append(e)

            vsb = []
            es0 = []
            for mp in range(4):
                ps = psD.tile([P, N], f32, tag="mm")
                for half in range(2):
                    mch = mp * 2 + half
                    for k in range(8):
                        nc.tensor.matmul(
                            ps[:, half * 512:(half + 1) * 512],
                            lhsT=cT[:, k, mch * P:(mch + 1) * P],
                            rhs=wv_sb[:, k, :],
                            start=(k == 0),
                            stop=(k == 7),
                        )
                for half in range(2):
                    # 64 ones-columns: the attn@v matmul replicates the
                    # softmax denominator across PSUM rows 64-127, so the
                    # partition broadcast of 1/denom costs nothing
                    vt = vpool.tile([P, 8, 2 * DH], bf16, tag="v")
                    # on DVE: keeps the Act queue free for the es0 exps that
                    # pace the dots0 psum ring
                    nc.vector.tensor_copy(
                        out=vt[:, :, 0:DH],
                        in_=ps[:, half * 512:(half + 1) * 512].rearrange(
                            "p (h d) -> p h d", d=DH
                        ),
                    )
                    nc.vector.memset(vt[:, :, DH:2 * DH], 1.0)
                    vsb.append(vt)
                dots_exp0_mch(2 * mp, es0)
                dots_exp0_mch(2 * mp + 1, es0)

        # ---- phase B: attention (bigT space now free)
        rcp = ctx.enter_context(tc.tile_pool(name="rcp", bufs=4))
        rbp = ctx.enter_context(tc.tile_pool(name="rbp", bufs=4))
        drp = ctx.enter_context(tc.tile_pool(name="drp", bufs=4, space="DRAM"))
        opool = ctx.enter_context(tc.tile_pool(name="opool", bufs=6))

        aoT = qk.tile([P, 4, N], bf16, tag="aoT")

        wo_sb = wpool.tile([P, 4, DIM], bf16, tag="w")
        for k in range(4):
            nc.sync.dma_start(wo_sb[:, k, :], wo[k * P:(k + 1) * P, :])

        def denom_normalize(h, pos):
            # PSUM rows 64-127 already hold the denominator replicated (ones
            # columns of v): move to sbuf, reciprocal, normalize.  All SBUF
            # operand pairs share start partitions.
            t2, r0 = h // 2, (h % 2) * 64
            rb = rbp.tile([P, N], f32, tag="rb")
            for ns in range(2):
                nsl = slice(ns * 512, (ns + 1) * 512)
                with nc.allow_low_precision(reason="softmax denom recip"):
                    nc.vector.reciprocal(
                        out=rb[r0:r0 + 64, nsl], in_=pos[ns][DH:2 * DH, :]
                    )
                nc.vector.tensor_mul(
                    out=aoT[r0:r0 + 64, t2, nsl],
                    in0=pos[ns][0:DH, :],
                    in1=rb[r0:r0 + 64, nsl],
                )

        # Main attention loop.  dots(h+1) and attn_v(h) are interleaved at
        # chunk granularity: the dots matmuls are gated by the exp-paced psD
        # ring, and the in-order PE queue would otherwise head-block the
        # (dependency-free) attn_v matmuls behind them.
        o1pool = ctx.enter_context(tc.tile_pool(name="o1pool", bufs=4))
        o1 = []
        es_cur = es0
        for h in range(8):
            if h < 7:
                t2, r0 = (h + 1) // 2, ((h + 1) % 2) * 64
                qh = qT[r0:r0 + 64, t2, :]
                kh = kT[r0:r0 + 64, t2, :]
            es_next = []
            pos = [psV.tile([P, 512], f32, tag="av", name=f"po{_i}")
                   for _i in range(2)]
            for mch in range(8):
                for ns in range(2):
                    nc.tensor.matmul(
                        pos[ns][:],
                        lhsT=vsb[mch][:, h, :],
                        rhs=es_cur[mch][:, ns * 512:(ns + 1) * 512],
                        start=(mch == 0),
                        stop=(mch == 7),
                    )
                if h < 7:
                    psd = psD.tile([P, N], f32, tag="mm")
                    for ns in range(2):
                        nc.tensor.matmul(
                            psd[:, ns * 512:(ns + 1) * 512],
                            lhsT=kh[:, mch * P:(mch + 1) * P],
                            rhs=qh[:, ns * 512:(ns + 1) * 512],
                            start=True,
                            stop=True,
                        )
                    e = epool.tile([P, N], bf16, tag="e")
                    nc.scalar.activation(e[:], psd[:], Exp, scale=SCALE)
                    es_next.append(e)
            denom_normalize(h, pos)
            es_cur = es_next
            # fold the kc=0,1 half of the final projection for nch 0-3 into
            # iters 4-5: psV has two spare ring slots there and the iters are
            # exp-paced with ~1.5us of PE slack
            if h in (4, 5):
                for nch in (2 * (h - 4), 2 * (h - 4) + 1):
                    o1t = o1pool.tile([P, N], f32, tag="o1",
                                      name=f"o1_{nch}")
                    for cc in range(2):
                        pf1 = psV.tile([P, 512], f32, tag="av")
                        for kc in range(2):
                            nc.tensor.matmul(
                                pf1[:],
                                lhsT=aoT[:, kc, nch * P:(nch + 1) * P],
                                rhs=wo_sb[:, kc, cc * 512:(cc + 1) * 512],
                                start=(kc == 0),
                                stop=(kc == 1),
                            )
                        nc.vector.tensor_copy(
                            out=o1t[:, cc * 512:(cc + 1) * 512], in_=pf1[:]
                        )
                    o1.append(o1t)

        # ---- final projection.  psD ring is free immediately (unlike psV,
        # whose last slots wait on norm(7)); only the kc=3 matmuls depend on
        # the last head's normalize chain.  Folded chunks (kc 2,3 + add of
        # the phase-B partial) alternate with full chunks so the DVE adds
        # hide behind the full chunks' matmul time.
        # Tiny Copy first: absorbs the Exp->Copy activation-table reload
        # while the PE is still on the first output chunk.
        warm = opool.tile([P, 8], f32, tag="warm")
        nc.scalar.activation(warm[:], cos_sb[:, 0:8], Copy)
        for nch in (0, 4, 1, 5, 2, 6, 3, 7):
            folded = nch < 4
            ot = opool.tile([P, N], bf16, tag="o")
            if folded:
                # folded chunks run through the psV ring (free in phase C):
                # no contention with the unfolded chunks' psD ring
                for cc in range(2):
                    ql = slice(cc * 512, (cc + 1) * 512)
                    pfh = psV.tile([P, 512], f32, tag="av")
                    for kc in range(2, 4):
                        nc.tensor.matmul(
                            pfh[:],
                            lhsT=aoT[:, kc, nch * P:(nch + 1) * P],
                            rhs=wo_sb[:, kc, cc * 512:(cc + 1) * 512],
                            start=(kc == 2),
                            stop=(kc == 3),
                        )
                    nc.vector.tensor_add(
                        out=ot[:, ql], in0=pfh[:], in1=o1[nch][:, ql]
                    )
                nc.sync.dma_start(out[nch * P:(nch + 1) * P, :], ot[:])
            else:
                # last chunk only: cc-outer so each 512-half's accumulation
                # group closes after its own 4 matmuls and the first
                # half-copy starts ~0.85us before the chunk's last matmul
                # (for earlier chunks kc-outer defers the norm(7)-gated kc=3)
                pf = psD.tile([P, N], f32, tag="mm")
                loops = ([(cc, kc) for cc in range(2) for kc in range(4)]
                         if nch == 7 else
                         [(cc, kc) for kc in range(4) for cc in range(2)])
                for cc, kc in loops:
                    nc.tensor.matmul(
                        pf[:, cc * 512:(cc + 1) * 512],
                        lhsT=aoT[:, kc, nch * P:(nch + 1) * P],
                        rhs=wo_sb[:, kc, cc * 512:(cc + 1) * 512],
                        start=(kc == 0),
                        stop=(kc == 3),
                    )
                if nch == 7:
                    # last chunk: per-half copies on the (now idle) Act
                    # engine with per-half stores — lowest drain latency
                    for q in range(2):
                        ql = slice(q * 512, (q + 1) * 512)
                        nc.scalar.activation(ot[:, ql], pf[:, ql], Copy)
                        nc.sync.dma_start(
                            out[nch * P:(nch + 1) * P, ql], ot[:, ql]
                        )
                else:
                    nc.scalar.activation(ot[:, 0:512], pf[:, 0:512], Copy)
                    nc.vector.tensor_copy(
                        out=ot[:, 512:1024], in_=pf[:, 512:1024]
                    )
                    nc.sync.dma_start(out[nch * P:(nch + 1) * P, :], ot[:])

    nc.compile()
    return nc


def _get_program():
    if "nc" not in _CACHE:
        _CACHE["nc"] = _build_program()
    return _CACHE["nc"]


def make_in_maps(x, context, rotary_pos, Wq, Wkv, Wout):
    from ml_dtypes import bfloat16

    x = np.asarray(x, dtype=np.float32)
    context = np.asarray(context, dtype=np.float32)
    rotary_pos = np.asarray(rotary_pos, dtype=np.float32)
    Wq = np.asarray(Wq, dtype=np.float32)
    Wkv = np.asarray(Wkv, dtype=np.float32)
    Wout = np.asarray(Wout, dtype=np.float32)

    def b16(a):
        return np.ascontiguousarray(a).astype(bfloat16)

    cosT = np.cos(rotary_pos).T  # [64, 1024]
    sinT = np.sin(rotary_pos).T
    sin_signed = np.concatenate([-sinT[:32], sinT[32:]], axis=0)
    cos2 = b16(np.vstack([cosT, cosT]))
    sin2 = b16(np.vstack([sin_signed, sin_signed]))

    in_maps = []
    for core in range(8):
        b, g = core // 2, core % 2
        cs = slice(g * ISH, (g + 1) * ISH)
        in_maps.append({
            "xbT": b16(x[b].T),
            "cxT": b16(context[b].T),
            "wq": b16(Wq[:, cs]),
            "wk": b16(Wkv[:, g * ISH:(g + 1) * ISH]),
            "wv": b16(Wkv[:, H * DH + g * ISH:H * DH + (g + 1) * ISH]),
            "wo": b16(Wout[cs, :]),
            "cos2": cos2,
            "sin2": sin2,
        })
    return in_maps


def kernel(x, context, mask, context_mask, rotary_pos, Wq, Wkv, Wout, bout):
    global _LAST_EXEC_NS
    from concourse.bass_utils import run_bass_kernel_spmd

    nc = _get_program()
    in_maps = make_in_maps(x, context, rotary_pos, Wq, Wkv, Wout)

    trace = bool(os.environ.get("BASS_KERNEL_TRACE"))
    res = run_bass_kernel_spmd(nc, in_maps, core_ids=list(range(8)), trace=trace)
    _LAST_EXEC_NS = res.exec_time_ns
    _CACHE["last_results"] = res

    bout = np.asarray(bout, dtype=np.float32)
    full = np.empty((B, N, DIM), dtype=np.float32)
    for b in range(B):
        full[b] = (
            res.results[2 * b]["out"].astype(np.float32)
            + res.results[2 * b + 1]["out"].astype(np.float32)
            + bout
        )
    return full


# revision 127
# speedup vs baseline: 1.0119x; 1.0011x over previous
"""CrossAttention Trainium2 kernel (bf16 pipeline).

Problem: nn_CrossAttention (B=4, N=M=1024, DIM=CTX_DIM=1024, H=16, DH=64).

Sharding: 8 cores = batch (4) x head-group (2 groups of 8 heads).
Each core computes, for its (b, g):
    q = rope(x[b] @ Wq[:, g])
    k = rope(context[b] @ Wk[:, g]);  v = context[b] @ Wv[:, g]
    attn = softmax(q k^T / sqrt(dh))     (mask is all-ones by construction)
    partial_out[b,g] = (attn @ v) @ Wout[g, :]
Host transposes x/context per batch and casts everything to bf16; it sums the
two head-group partials per batch and adds bout.

All matmuls run in bf16 (fp32 PSUM accumulation).  bf16 moving data streams at
1 cycle/row and the separate Ldweights instructions keep the PE p-state ramp
warm.  Dots/projection PSUM tiles are [128, 1024] (2 banks) so the Activation
engine amortizes its access latency over 1024-wide exp/cast chunks.

Engine assignment:
    PE    : all matmuls (projections, dots, attn@v, final)
    Act   : psum->bf16 casts feeding rope, exp(dots) -> es bf16, half the
            final copies
    DVE   : rope muls (bf16 SBUF, 2x perf mode), denominator reciprocal
            (reads PSUM rows 64-127 directly), normalize-mult fused with the
            psum->sbuf move of attn@v outputs, half the final copies
    Pool  : wq/cos/sin/wk loads via software DGE (fastest first-chunk
            latency; engine otherwise idle)
    SP    : x/context/wv/wo loads, half the rope-rotation DMAs, stores

Key tricks:
  - rope's rotate_half is a partition permutation (p XOR 32): done by small
    SBUF->SBUF DMAs (2 on SP, 2 on Act per chunk), because DVE tensor-tensor
    ops require equal SBUF start partitions (walrus
    checkSBSameStartPartition) while DMA addresses partitions freely.
  - v carries 64 ones-COLUMNS, so the attn@v matmul replicates the softmax
    denominator across PSUM rows 64-127 for free (matmul cost is moving-rows
    only); normalization is then reciprocal + one mul per half, all
    same-start.
  - head 0's dots+exp are interleaved with the v projection so the Act
    engine is warm when the (Act-exp-bound) attention loop starts; in the
    loop, attn_v(h) chunks are issued before the exp-ring-gated dots(h+1)
    chunks so the in-order PE queue never head-blocks.
  - the kc=0,1 half of the final projection for output chunks 0-3 runs
    inside attention iters 4-5 (psV ring slack, exp-paced PE slack); phase C
    then alternates folded chunks (kc 2,3 + one wide DVE add of the partial)
    with full chunks so the adds hide behind matmul time.
  - gpsimd partition_broadcast and rearrange-split-partition DMA APs both
    break on real hardware despite passing CoreSim/TimelineSim -- avoided.
"""

import os
import numpy as np

B, N, M = 4, 1024, 1024
DIM = 1024
H, DH = 16, 64
ISH = 512  # inner shard per core (8 heads * 64)
SCALE = DH ** -0.5
P = 128

_CACHE = {}
_LAST_EXEC_NS = None


def _build_program():
    from contextlib import ExitStack

    import concourse.tile as tile
    from concourse import bacc, mybir

    f32 = mybir.dt.float32
    bf16 = mybir.dt.bfloat16
    Exp = mybir.ActivationFunctionType.Exp
    Copy = mybir.ActivationFunctionType.Copy

    nc = bacc.Bacc("TRN2", target_bir_lowering=False, debug=False, num_devices=8)

    xbT = nc.dram_tensor("xbT", [DIM, N], bf16, kind="ExternalInput").ap()
    cxT = nc.dram_tensor("cxT", [DIM, M], bf16, kind="ExternalInput").ap()
    wq = nc.dram_tensor("wq", [DIM, ISH], bf16, kind="ExternalInput").ap()
    wk = nc.dram_tensor("wk", [DIM, ISH], bf16, kind="ExternalInput").ap()
    wv = nc.dram_tensor("wv", [DIM, ISH], bf16, kind="ExternalInput").ap()
    wo = nc.dram_tensor("wo", [ISH, DIM], bf16, kind="ExternalInput").ap()
    cos2 = nc.dram_tensor("cos2", [P, N], bf16, kind="ExternalInput").ap()
    sin2 = nc.dram_tensor("sin2", [P, N], bf16, kind="ExternalInput").ap()
    out = nc.dram_tensor("out", [N, DIM], bf16, kind="ExternalOutput").ap()

    with tile.TileContext(nc) as tc, ExitStack() as ctx:
        const = ctx.enter_context(tc.tile_pool(name="const", bufs=1))
        wpool = ctx.enter_context(tc.tile_pool(name="wpool", bufs=2))
        qk = ctx.enter_context(tc.tile_pool(name="qk", bufs=1))
        vpool = ctx.enter_context(tc.tile_pool(name="vpool", bufs=8))
        tmpp = ctx.enter_context(tc.tile_pool(name="tmpp", bufs=8))

        wq_sb = wpool.tile([P, 8, ISH], bf16, tag="w")
        wk_sb = wpool.tile([P, 8, ISH], bf16, tag="w")
        wv_sb = wpool.tile([P, 8, ISH], bf16, tag="w")
        for k in range(8):
            nc.gpsimd.dma_start(wq_sb[:, k, :], wq[k * P:(k + 1) * P, :])
        cos_sb = const.tile([P, N], bf16, tag="cos")
        nc.gpsimd.dma_start(cos_sb[:], cos2)
        sin_sb = const.tile([P, N], bf16, tag="sin")
        nc.gpsimd.dma_start(sin_sb[:], sin2)
        for k in range(8):
            nc.gpsimd.dma_start(wk_sb[:, k, :], wk[k * P:(k + 1) * P, :])

        # ---- phase A: projections (xT/ctxT big tiles live only here)
        psAB = ctx.enter_context(ExitStack())
        psD = psAB.enter_context(tc.tile_pool(name="psD", bufs=2, space="PSUM"))
        psV = psAB.enter_context(tc.tile_pool(name="psV", bufs=4, space="PSUM"))
        epool = ctx.enter_context(tc.tile_pool(name="epool", bufs=16))
        with tc.tile_pool(name="bigT", bufs=2) as bigT:

            def project_rope(xT, w_sb, tag):
                dst = qk.tile([P, 4, N], bf16, tag=tag)
                for ic in range(4):
                    ps = psD.tile([P, N], f32, tag="mm")
                    for k in range(8):
                        for ns in range(2):
                            lt = w_sb[:, k, ic * P:(ic + 1) * P]
                            if w_sb is wq_sb and ic == 0 and k == 0:
                                lt = wq00[:]
                            nc.tensor.matmul(
                                ps[:, ns * 512:(ns + 1) * 512],
                                lhsT=lt,
                                rhs=xT[:, k, ns * 512:(ns + 1) * 512],
                                start=(k == 0),
                                stop=(k == 7),
                            )
                    qc = tmpp.tile([P, N], bf16, tag="qc")
                    nc.scalar.activation(qc[:], ps[:], Copy)
                    # rope: dst = qc * cos + rotate_half(qc) * sin_signed.
                    # The partition rotation (p -> p XOR 32) runs on the DMA
                    # engines: DVE tensor-tensor ops require equal SBUF start
                    # partitions (walrus checkSBSameStartPartition), and DMA
                    # addresses partitions freely.  Issue split across the SP
                    # and DVE queues to fit their sequencer budgets.
                    qcr = tmpp.tile([P, N], bf16, tag="qcr")
                    for blk in range(4):
                        d0 = blk * 32
                        s0 = (blk ^ 1) * 32
                        eng = nc.sync if blk % 2 == 0 else nc.scalar
                        eng.dma_start(
                            qcr[d0:d0 + 32, :], qc[s0:s0 + 32, :]
                        )
                    dsl = dst[:, ic, :]
                    nc.vector.tensor_mul(out=dsl, in0=qc[:], in1=cos_sb[:])
                    tmp = tmpp.tile([P, N], bf16, tag="tmp")
                    nc.vector.tensor_mul(out=tmp[:], in0=qcr[:], in1=sin_sb[:])
                    nc.vector.tensor_add(out=dsl, in0=dsl, in1=tmp[:])
                return dst

            # DMA issue plan: SP carries wq0 (fastest path for the first
            # matmul) then x/context/wv; Act queue carries wq1-7 in parallel
            # and is free for the rope casts by ~5us; Pool carries cos/sin/wk
            # (software DGE, idle engine).
            wq00 = bigT.tile([P, P], bf16, tag="wq00")
            nc.scalar.dma_start(wq00[:], wq[0:P, 0:P])
            xT = bigT.tile([P, 8, N], bf16, tag="bigT")
            for k in range(8):
                nc.sync.dma_start(xT[:, k, :], xbT[k * P:(k + 1) * P, :])
            cT = bigT.tile([P, 8, N], bf16, tag="bigT")
            for k in range(8):
                nc.sync.dma_start(cT[:, k, :], cxT[k * P:(k + 1) * P, :])
            for k in range(8):
                nc.sync.dma_start(wv_sb[:, k, :], wv[k * P:(k + 1) * P, :])
            qT = project_rope(xT, wq_sb, "qT")
            kT = project_rope(cT, wk_sb, "kT")

            def dots_exp0_mch(mch, es):
                # head 0's dots+exp through the (phase-A-idle) attention psum
                # ring, interleaved with the v projection so the Act engine
                # stays busy through phase A's tail
                e = epool.tile([P, N], bf16, tag="e")
                for ns in range(2):
                    psd = psV.tile([P, 512], f32, tag="av")
                    nc.tensor.matmul(
                        psd[:],
                        lhsT=kT[0:64, 0, mch * P:(mch + 1) * P],
                        rhs=qT[0:64, 0, ns * 512:(ns + 1) * 512],
                        start=True,
                        stop=True,
                    )
                    nc.scalar.activation(
                        e[:, ns * 512:(ns + 1) * 512], psd[:], Exp,
                        scale=SCALE,
                    )
                es.append(e)

            # vt tiles allocated upfront: the ones-columns memsets run at
            # program start on the idle DVE instead of inside the v-window
            vsb = [vpool.tile([P, 8, 2 * DH], bf16, tag="v", name=f"vt{_i}")
                   for _i in range(8)]
            for vt in vsb:
                nc.vector.memset(vt[:, :, DH:2 * DH], 1.0)
            es0 = []
            for mp in range(4):
                ps = psD.tile([P, N], f32, tag="mm")
                for half in range(2):
                    mch = mp * 2 + half
                    for k in range(8):
                        nc.tensor.matmul(
                            ps[:, half * 512:(half + 1) * 512],
                            lhsT=cT[:, k, mch * P:(mch + 1) * P],
                            rhs=wv_sb[:, k, :],
                            start=(k == 0),
                            stop=(k == 7),
                        )
                for half in range(2):
                    # 64 ones-columns (memset upfront): the attn@v matmul
                    # replicates the softmax denominator across PSUM rows
                    # 64-127, so the partition broadcast of 1/denom is free.
                    # Copy on DVE: keeps the Act queue clear for the es0
                    # exps that pace the dots0 psum ring.
                    vt = vsb[mp * 2 + half]
                    nc.vector.tensor_copy(
                        out=vt[:, :, 0:DH],
                        in_=ps[:, half * 512:(half + 1) * 512].rearrange(
                            "p (h d) -> p h d", d=DH
                        ),
                    )
                dots_exp0_mch(2 * mp, es0)
                dots_exp0_mch(2 * mp + 1, es0)

        # ---- phase B: attention (bigT space now free)
        rcp = ctx.enter_context(tc.tile_pool(name="rcp", bufs=4))
        rbp = ctx.enter_context(tc.tile_pool(name="rbp", bufs=4))
        drp = ctx.enter_context(tc.tile_pool(name="drp", bufs=4, space="DRAM"))
        opool = ctx.enter_context(tc.tile_pool(name="opool", bufs=6))

        aoT = qk.tile([P, 4, N], bf16, tag="aoT")

        wo_sb = wpool.tile([P, 4, DIM], bf16, tag="w")
        for k in range(4):
            nc.sync.dma_start(wo_sb[:, k, :], wo[k * P:(k + 1) * P, :])

        def denom_normalize(h, pos):
            # PSUM rows 64-127 already hold the denominator replicated (ones
            # columns of v): move to sbuf, reciprocal, normalize.  All SBUF
            # operand pairs share start partitions.
            t2, r0 = h // 2, (h % 2) * 64
            rb = rbp.tile([P, N], f32, tag="rb")
            for ns in range(2):
                nsl = slice(ns * 512, (ns + 1) * 512)
                with nc.allow_low_precision(reason="softmax denom recip"):
                    nc.vector.reciprocal(
                        out=rb[r0:r0 + 64, nsl], in_=pos[ns][DH:2 * DH, :]
                    )
                nc.vector.tensor_mul(
                    out=aoT[r0:r0 + 64, t2, nsl],
                    in0=pos[ns][0:DH, :],
                    in1=rb[r0:r0 + 64, nsl],
                )

        # Main attention loop.  dots(h+1) and attn_v(h) are interleaved at
        # chunk granularity: the dots matmuls are gated by the exp-paced psD
        # ring, and the in-order PE queue would otherwise head-block the
        # (dependency-free) attn_v matmuls behind them.
        o1pool = ctx.enter_context(tc.tile_pool(name="o1pool", bufs=4))
        o1 = []
        es_cur = es0
        for h in range(8):
            if h < 7:
                t2, r0 = (h + 1) // 2, ((h + 1) % 2) * 64
                qh = qT[r0:r0 + 64, t2, :]
                kh = kT[r0:r0 + 64, t2, :]
            es_next = []
            pos = [psV.tile([P, 512], f32, tag="av", name=f"po{_i}")
                   for _i in range(2)]
            for mch in range(8):
                for ns in range(2):
                    nc.tensor.matmul(
                        pos[ns][:],
                        lhsT=vsb[mch][:, h, :],
                        rhs=es_cur[mch][:, ns * 512:(ns + 1) * 512],
                        start=(mch == 0),
                        stop=(mch == 7),
                    )
                if h < 7:
                    psd = psD.tile([P, N], f32, tag="mm")
                    for ns in range(2):
                        nc.tensor.matmul(
                            psd[:, ns * 512:(ns + 1) * 512],
                            lhsT=kh[:, mch * P:(mch + 1) * P],
                            rhs=qh[:, ns * 512:(ns + 1) * 512],
                            start=True,
                            stop=True,
                        )
                    e = epool.tile([P, N], bf16, tag="e")
                    nc.scalar.activation(e[:], psd[:], Exp, scale=SCALE)
                    es_next.append(e)
            denom_normalize(h, pos)
            es_cur = es_next
            # fold the kc=0,1 half of the final projection for nch 0-3 into
            # iters 4-5: psV has two spare ring slots there and the iters are
            # exp-paced with ~1.5us of PE slack
            if h in (4, 5):
                for nch in (2 * (h - 4), 2 * (h - 4) + 1):
                    o1t = o1pool.tile([P, N], f32, tag="o1",
                                      name=f"o1_{nch}")
                    for cc in range(2):
                        pf1 = psV.tile([P, 512], f32, tag="av")
                        for kc in range(2):
                            nc.tensor.matmul(
                                pf1[:],
                                lhsT=aoT[:, kc, nch * P:(nch + 1) * P],
                                rhs=wo_sb[:, kc, cc * 512:(cc + 1) * 512],
                                start=(kc == 0),
                                stop=(kc == 1),
                            )
                        nc.vector.tensor_copy(
                            out=o1t[:, cc * 512:(cc + 1) * 512], in_=pf1[:]
                        )
                    o1.append(o1t)

        # ---- final projection.  psD ring is free immediately (unlike psV,
        # whose last slots wait on norm(7)); only the kc=3 matmuls depend on
        # the last head's normalize chain.  Folded chunks (kc 2,3 + add of
        # the phase-B partial) alternate with full chunks so the DVE adds
        # hide behind the full chunks' matmul time.
        # Tiny Copy first: absorbs the Exp->Copy activation-table reload
        # while the PE is still on the first output chunk.
        warm = opool.tile([P, 8], f32, tag="warm")
        nc.scalar.activation(warm[:], cos_sb[:, 0:8], Copy)
        for nch in (0, 4, 1, 5, 2, 6, 3, 7):
            folded = nch < 4
            ot = opool.tile([P, N], bf16, tag="o")
            if folded:
                # folded chunks run through the psV ring (free in phase C):
                # no contention with the unfolded chunks' psD ring
                for cc in range(2):
                    ql = slice(cc * 512, (cc + 1) * 512)
                    pfh = psV.tile([P, 512], f32, tag="av")
                    for kc in range(2, 4):
                        nc.tensor.matmul(
                            pfh[:],
                            lhsT=aoT[:, kc, nch * P:(nch + 1) * P],
                            rhs=wo_sb[:, kc, cc * 512:(cc + 1) * 512],
                            start=(kc == 2),
                            stop=(kc == 3),
                        )
                    nc.vector.tensor_add(
                        out=ot[:, ql], in0=pfh[:], in1=o1[nch][:, ql]
                    )
                nc.sync.dma_start(out[nch * P:(nch + 1) * P, :], ot[:])
            else:
                # last chunk only: cc-outer so each 512-half's accumulation
                # group closes after its own 4 matmuls and the first
                # half-copy starts ~0.85us before the chunk's last matmul
                # (for earlier chunks kc-outer defers the norm(7)-gated kc=3)
                pf = psD.tile([P, N], f32, tag="mm")
                loops = ([(cc, kc) for cc in range(2) for kc in range(4)]
                         if nch == 7 else
                         [(cc, kc) for kc in range(4) for cc in range(2)])
                for cc, kc in loops:
                    nc.tensor.matmul(
                        pf[:, cc * 512:(cc + 1) * 512],
                        lhsT=aoT[:, kc, nch * P:(nch + 1) * P],
                        rhs=wo_sb[:, kc, cc * 512:(cc + 1) * 512],
                        start=(kc == 0),
                        stop=(kc == 3),
                    )
                if nch == 7:
                    # last chunk: per-half copies on the (now idle) Act
                    # engine with per-half stores — lowest drain latency
                    for q in range(2):
                        ql = slice(q * 512, (q + 1) * 512)
                        nc.scalar.activation(ot[:, ql], pf[:, ql], Copy)
                        nc.sync.dma_start(
                            out[nch * P:(nch + 1) * P, ql], ot[:, ql]
                        )
                else:
                    nc.scalar.activation(ot[:, 0:512], pf[:, 0:512], Copy)
                    nc.vector.tensor_copy(
                        out=ot[:, 512:1024], in_=pf[:, 512:1024]
                    )
                    nc.sync.dma_start(out[nch * P:(nch + 1) * P, :], ot[:])

    nc.compile()
    return nc


def _get_program():
    if "nc" not in _CACHE:
        _CACHE["nc"] = _build_program()
    return _CACHE["nc"]


def make_in_maps(x, context, rotary_pos, Wq, Wkv, Wout):
    from ml_dtypes import bfloat16

    x = np.asarray(x, dtype=np.float32)
    context = np.asarray(context, dtype=np.float32)
    rotary_pos = np.asarray(rotary_pos, dtype=np.float32)
    Wq = np.asarray(Wq, dtype=np.float32)
    Wkv = np.asarray(Wkv, dtype=np.float32)
    Wout = np.asarray(Wout, dtype=np.float32)

    def b16(a):
        return np.ascontiguousarray(a).astype(bfloat16)

    cosT = np.cos(rotary_pos).T  # [64, 1024]
    sinT = np.sin(rotary_pos).T
    sin_signed = np.concatenate([-sinT[:32], sinT[32:]], axis=0)
    cos2 = b16(np.vstack([cosT, cosT]))
    sin2 = b16(np.vstack([sin_signed, sin_signed]))

    in_maps = []
    for core in range(8):
        b, g = core // 2, core % 2
        cs = slice(g * ISH, (g + 1) * ISH)
        in_maps.append({
            "xbT": b16(x[b].T),
            "cxT": b16(context[b].T),
            "wq": b16(Wq[:, cs]),
            "wk": b16(Wkv[:, g * ISH:(g + 1) * ISH]),
            "wv": b16(Wkv[:, H * DH + g * ISH:H * DH + (g + 1) * ISH]),
            "wo": b16(Wout[cs, :]),
            "cos2": cos2,
            "sin2": sin2,
        })
    return in_maps


def kernel(x, context, mask, context_mask, rotary_pos, Wq, Wkv, Wout, bout):
    global _LAST_EXEC_NS
    from concourse.bass_utils import run_bass_kernel_spmd

    nc = _get_program()
    in_maps = make_in_maps(x, context, rotary_pos, Wq, Wkv, Wout)

    trace = bool(os.environ.get("BASS_KERNEL_TRACE"))
    res = run_bass_kernel_spmd(nc, in_maps, core_ids=list(range(8)), trace=trace)
    _LAST_EXEC_NS = res.exec_time_ns
    _CACHE["last_results"] = res

    bout = np.asarray(bout, dtype=np.float32)
    full = np.empty((B, N, DIM), dtype=np.float32)
    for b in range(B):
        full[b] = (
            res.results[2 * b]["out"].astype(np.float32)
            + res.results[2 * b + 1]["out"].astype(np.float32)
            + bout
        )
    return full


# revision 128
# speedup vs baseline: 1.0151x; 1.0032x over previous
"""CrossAttention Trainium2 kernel (bf16 pipeline).

Problem: nn_CrossAttention (B=4, N=M=1024, DIM=CTX_DIM=1024, H=16, DH=64).

Sharding: 8 cores = batch (4) x head-group (2 groups of 8 heads).
Each core computes, for its (b, g):
    q = rope(x[b] @ Wq[:, g])
    k = rope(context[b] @ Wk[:, g]);  v = context[b] @ Wv[:, g]
    attn = softmax(q k^T / sqrt(dh))     (mask is all-ones by construction)
    partial_out[b,g] = (attn @ v) @ Wout[g, :]
Host transposes x/context per batch and casts everything to bf16; it sums the
two head-group partials per batch and adds bout.

All matmuls run in bf16 (fp32 PSUM accumulation).  bf16 moving data streams at
1 cycle/row and the separate Ldweights instructions keep the PE p-state ramp
warm.  Dots/projection PSUM tiles are [128, 1024] (2 banks) so the Activation
engine amortizes its access latency over 1024-wide exp/cast chunks.

Engine assignment:
    PE    : all matmuls (projections, dots, attn@v, final)
    Act   : psum->bf16 casts feeding rope, exp(dots) -> es bf16, half the
            final copies
    DVE   : rope muls (bf16 SBUF, 2x perf mode), denominator reciprocal
            (reads PSUM rows 64-127 directly), normalize-mult fused with the
            psum->sbuf move of attn@v outputs, half the final copies
    Pool  : wq/cos/sin/wk loads via software DGE (fastest first-chunk
            latency; engine otherwise idle)
    SP    : x/context/wv/wo loads, half the rope-rotation DMAs, stores

Key tricks:
  - rope's rotate_half is a partition permutation (p XOR 32): done by small
    SBUF->SBUF DMAs (2 on SP, 2 on Act per chunk), because DVE tensor-tensor
    ops require equal SBUF start partitions (walrus
    checkSBSameStartPartition) while DMA addresses partitions freely.
  - v carries 64 ones-COLUMNS, so the attn@v matmul replicates the softmax
    denominator across PSUM rows 64-127 for free (matmul cost is moving-rows
    only); normalization is then reciprocal + one mul per half, all
    same-start.
  - head 0's dots+exp are interleaved with the v projection so the Act
    engine is warm when the (Act-exp-bound) attention loop starts; in the
    loop, attn_v(h) chunks are issued before the exp-ring-gated dots(h+1)
    chunks so the in-order PE queue never head-blocks.
  - the kc=0,1 half of the final projection for output chunks 0-3 runs
    inside attention iters 4-5 (psV ring slack, exp-paced PE slack); phase C
    then alternates folded chunks (kc 2,3 + one wide DVE add of the partial)
    with full chunks so the adds hide behind matmul time.
  - gpsimd partition_broadcast and rearrange-split-partition DMA APs both
    break on real hardware despite passing CoreSim/TimelineSim -- avoided.
"""

import os
import numpy as np

B, N, M = 4, 1024, 1024
DIM = 1024
H, DH = 16, 64
ISH = 512  # inner shard per core (8 heads * 64)
SCALE = DH ** -0.5
P = 128

_CACHE = {}
_LAST_EXEC_NS = None


def _build_program():
    from contextlib import ExitStack

    import concourse.tile as tile
    from concourse import bacc, mybir

    f32 = mybir.dt.float32
    bf16 = mybir.dt.bfloat16
    Exp = mybir.ActivationFunctionType.Exp
    Copy = mybir.ActivationFunctionType.Copy

    nc = bacc.Bacc("TRN2", target_bir_lowering=False, debug=False, num_devices=8)

    xbT = nc.dram_tensor("xbT", [DIM, N], bf16, kind="ExternalInput").ap()
    cxT = nc.dram_tensor("cxT", [DIM, M], bf16, kind="ExternalInput").ap()
    wq = nc.dram_tensor("wq", [DIM, ISH], bf16, kind="ExternalInput").ap()
    wk = nc.dram_tensor("wk", [DIM, ISH], bf16, kind="ExternalInput").ap()
    wv = nc.dram_tensor("wv", [DIM, ISH], bf16, kind="ExternalInput").ap()
    wo = nc.dram_tensor("wo", [ISH, DIM], bf16, kind="ExternalInput").ap()
    cos2 = nc.dram_tensor("cos2", [P, N], bf16, kind="ExternalInput").ap()
    sin2 = nc.dram_tensor("sin2", [P, N], bf16, kind="ExternalInput").ap()
    out = nc.dram_tensor("out", [N, DIM], bf16, kind="ExternalOutput").ap()

    with tile.TileContext(nc) as tc, ExitStack() as ctx:
        const = ctx.enter_context(tc.tile_pool(name="const", bufs=1))
        wpool = ctx.enter_context(tc.tile_pool(name="wpool", bufs=2))
        qk = ctx.enter_context(tc.tile_pool(name="qk", bufs=1))
        vpool = ctx.enter_context(tc.tile_pool(name="vpool", bufs=8))
        tmpp = ctx.enter_context(tc.tile_pool(name="tmpp", bufs=8))

        wq_sb = wpool.tile([P, 8, ISH], bf16, tag="w")
        wk_sb = wpool.tile([P, 8, ISH], bf16, tag="w")
        wv_sb = wpool.tile([P, 8, ISH], bf16, tag="w")
        for k in range(8):
            nc.gpsimd.dma_start(wq_sb[:, k, :], wq[k * P:(k + 1) * P, :])
        cos_sb = const.tile([P, N], bf16, tag="cos")
        nc.gpsimd.dma_start(cos_sb[:], cos2)
        sin_sb = const.tile([P, N], bf16, tag="sin")
        nc.gpsimd.dma_start(sin_sb[:], sin2)
        for k in range(8):
            nc.gpsimd.dma_start(wk_sb[:, k, :], wk[k * P:(k + 1) * P, :])

        # ---- phase A: projections (xT/ctxT big tiles live only here)
        psAB = ctx.enter_context(ExitStack())
        psD = psAB.enter_context(tc.tile_pool(name="psD", bufs=2, space="PSUM"))
        psV = psAB.enter_context(tc.tile_pool(name="psV", bufs=4, space="PSUM"))
        epool = ctx.enter_context(tc.tile_pool(name="epool", bufs=16))
        with tc.tile_pool(name="bigT", bufs=2) as bigT:

            def project_rope(xT, w_sb, tag):
                dst = qk.tile([P, 4, N], bf16, tag=tag)
                for ic in range(4):
                    ps = psD.tile([P, N], f32, tag="mm")
                    for k in range(8):
                        for ns in range(2):
                            lt = w_sb[:, k, ic * P:(ic + 1) * P]
                            if w_sb is wq_sb and ic == 0 and k == 0:
                                lt = wq00[:]
                            nc.tensor.matmul(
                                ps[:, ns * 512:(ns + 1) * 512],
                                lhsT=lt,
                                rhs=xT[:, k, ns * 512:(ns + 1) * 512],
                                start=(k == 0),
                                stop=(k == 7),
                            )
                    qc = tmpp.tile([P, N], bf16, tag="qc")
                    nc.scalar.activation(qc[:], ps[:], Copy)
                    # rope: dst = qc * cos + rotate_half(qc) * sin_signed.
                    # The partition rotation (p -> p XOR 32) runs on the DMA
                    # engines: DVE tensor-tensor ops require equal SBUF start
                    # partitions (walrus checkSBSameStartPartition), and DMA
                    # addresses partitions freely.  Issue split across the SP
                    # and DVE queues to fit their sequencer budgets.
                    qcr = tmpp.tile([P, N], bf16, tag="qcr")
                    for blk in range(4):
                        d0 = blk * 32
                        s0 = (blk ^ 1) * 32
                        eng = (nc.sync if (blk % 2 == 0 or w_sb is wk_sb)
                               else nc.scalar)
                        eng.dma_start(
                            qcr[d0:d0 + 32, :], qc[s0:s0 + 32, :]
                        )
                    dsl = dst[:, ic, :]
                    nc.vector.tensor_mul(out=dsl, in0=qc[:], in1=cos_sb[:])
                    tmp = tmpp.tile([P, N], bf16, tag="tmp")
                    nc.vector.tensor_mul(out=tmp[:], in0=qcr[:], in1=sin_sb[:])
                    nc.vector.tensor_add(out=dsl, in0=dsl, in1=tmp[:])
                return dst

            # DMA issue plan: SP carries wq0 (fastest path for the first
            # matmul) then x/context/wv; Act queue carries wq1-7 in parallel
            # and is free for the rope casts by ~5us; Pool carries cos/sin/wk
            # (software DGE, idle engine).
            wq00 = bigT.tile([P, P], bf16, tag="wq00")
            nc.scalar.dma_start(wq00[:], wq[0:P, 0:P])
            xT = bigT.tile([P, 8, N], bf16, tag="bigT")
            for k in range(8):
                nc.sync.dma_start(xT[:, k, :], xbT[k * P:(k + 1) * P, :])
            cT = bigT.tile([P, 8, N], bf16, tag="bigT")
            for k in range(8):
                nc.sync.dma_start(cT[:, k, :], cxT[k * P:(k + 1) * P, :])
            for k in range(8):
                nc.sync.dma_start(wv_sb[:, k, :], wv[k * P:(k + 1) * P, :])
            qT = project_rope(xT, wq_sb, "qT")
            kT = project_rope(cT, wk_sb, "kT")

            def dots_exp0_mch(mch, es):
                # head 0's dots+exp through the (phase-A-idle) attention psum
                # ring, interleaved with the v projection so the Act engine
                # stays busy through phase A's tail
                e = epool.tile([P, N], bf16, tag="e")
                for ns in range(2):
                    psd = psV.tile([P, 512], f32, tag="av")
                    nc.tensor.matmul(
                        psd[:],
                        lhsT=kT[0:64, 0, mch * P:(mch + 1) * P],
                        rhs=qT[0:64, 0, ns * 512:(ns + 1) * 512],
                        start=True,
                        stop=True,
                    )
                    nc.scalar.activation(
                        e[:, ns * 512:(ns + 1) * 512], psd[:], Exp,
                        scale=SCALE,
                    )
                es.append(e)

            # vt tiles allocated upfront: the ones-columns memsets run at
            # program start on the idle DVE instead of inside the v-window
            vsb = [vpool.tile([P, 8, 2 * DH], bf16, tag="v", name=f"vt{_i}")
                   for _i in range(8)]
            for vt in vsb:
                nc.vector.memset(vt[:, :, DH:2 * DH], 1.0)
            es0 = []
            for mp in range(4):
                ps = psD.tile([P, N], f32, tag="mm")
                for half in range(2):
                    mch = mp * 2 + half
                    for k in range(8):
                        nc.tensor.matmul(
                            ps[:, half * 512:(half + 1) * 512],
                            lhsT=cT[:, k, mch * P:(mch + 1) * P],
                            rhs=wv_sb[:, k, :],
                            start=(k == 0),
                            stop=(k == 7),
                        )
                for half in range(2):
                    # 64 ones-columns (memset upfront): the attn@v matmul
                    # replicates the softmax denominator across PSUM rows
                    # 64-127, so the partition broadcast of 1/denom is free.
                    # Copy on DVE: keeps the Act queue clear for the es0
                    # exps that pace the dots0 psum ring.
                    vt = vsb[mp * 2 + half]
                    nc.vector.tensor_copy(
                        out=vt[:, :, 0:DH],
                        in_=ps[:, half * 512:(half + 1) * 512].rearrange(
                            "p (h d) -> p h d", d=DH
                        ),
                    )
                dots_exp0_mch(2 * mp, es0)
                dots_exp0_mch(2 * mp + 1, es0)

        # ---- phase B: attention (bigT space now free)
        rcp = ctx.enter_context(tc.tile_pool(name="rcp", bufs=4))
        rbp = ctx.enter_context(tc.tile_pool(name="rbp", bufs=4))
        drp = ctx.enter_context(tc.tile_pool(name="drp", bufs=4, space="DRAM"))
        opool = ctx.enter_context(tc.tile_pool(name="opool", bufs=6))

        aoT = qk.tile([P, 4, N], bf16, tag="aoT")

        wo_sb = wpool.tile([P, 4, DIM], bf16, tag="w")
        for k in range(4):
            nc.sync.dma_start(wo_sb[:, k, :], wo[k * P:(k + 1) * P, :])

        def denom_normalize(h, pos):
            # PSUM rows 64-127 already hold the denominator replicated (ones
            # columns of v): move to sbuf, reciprocal, normalize.  All SBUF
            # operand pairs share start partitions.
            t2, r0 = h // 2, (h % 2) * 64
            rb = rbp.tile([P, N], f32, tag="rb")
            for ns in range(2):
                nsl = slice(ns * 512, (ns + 1) * 512)
                with nc.allow_low_precision(reason="softmax denom recip"):
                    nc.vector.reciprocal(
                        out=rb[r0:r0 + 64, nsl], in_=pos[ns][DH:2 * DH, :]
                    )
                nc.vector.tensor_mul(
                    out=aoT[r0:r0 + 64, t2, nsl],
                    in0=pos[ns][0:DH, :],
                    in1=rb[r0:r0 + 64, nsl],
                )

        # Main attention loop.  dots(h+1) and attn_v(h) are interleaved at
        # chunk granularity: the dots matmuls are gated by the exp-paced psD
        # ring, and the in-order PE queue would otherwise head-block the
        # (dependency-free) attn_v matmuls behind them.
        o1pool = ctx.enter_context(tc.tile_pool(name="o1pool", bufs=4))
        o1 = []
        es_cur = es0
        for h in range(8):
            if h < 7:
                t2, r0 = (h + 1) // 2, ((h + 1) % 2) * 64
                qh = qT[r0:r0 + 64, t2, :]
                kh = kT[r0:r0 + 64, t2, :]
            es_next = []
            pos = [psV.tile([P, 512], f32, tag="av", name=f"po{_i}")
                   for _i in range(2)]
            for mch in range(8):
                for ns in range(2):
                    nc.tensor.matmul(
                        pos[ns][:],
                        lhsT=vsb[mch][:, h, :],
                        rhs=es_cur[mch][:, ns * 512:(ns + 1) * 512],
                        start=(mch == 0),
                        stop=(mch == 7),
                    )
                if h < 7:
                    psd = psD.tile([P, N], f32, tag="mm")
                    for ns in range(2):
                        nc.tensor.matmul(
                            psd[:, ns * 512:(ns + 1) * 512],
                            lhsT=kh[:, mch * P:(mch + 1) * P],
                            rhs=qh[:, ns * 512:(ns + 1) * 512],
                            start=True,
                            stop=True,
                        )
                    e = epool.tile([P, N], bf16, tag="e")
                    nc.scalar.activation(e[:], psd[:], Exp, scale=SCALE)
                    es_next.append(e)
            denom_normalize(h, pos)
            es_cur = es_next
            # fold the kc=0,1 half of the final projection for nch 0-3 into
            # iters 4-5: psV has two spare ring slots there and the iters are
            # exp-paced with ~1.5us of PE slack
            if h in (4, 5):
                for nch in (2 * (h - 4), 2 * (h - 4) + 1):
                    o1t = o1pool.tile([P, N], f32, tag="o1",
                                      name=f"o1_{nch}")
                    for cc in range(2):
                        pf1 = psV.tile([P, 512], f32, tag="av")
                        for kc in range(2):
                            nc.tensor.matmul(
                                pf1[:],
                                lhsT=aoT[:, kc, nch * P:(nch + 1) * P],
                                rhs=wo_sb[:, kc, cc * 512:(cc + 1) * 512],
                                start=(kc == 0),
                                stop=(kc == 1),
                            )
                        nc.vector.tensor_copy(
                            out=o1t[:, cc * 512:(cc + 1) * 512], in_=pf1[:]
                        )
                    o1.append(o1t)

        # ---- final projection.  psD ring is free immediately (unlike psV,
        # whose last slots wait on norm(7)); only the kc=3 matmuls depend on
        # the last head's normalize chain.  Folded chunks (kc 2,3 + add of
        # the phase-B partial) alternate with full chunks so the DVE adds
        # hide behind the full chunks' matmul time.
        # Tiny Copy first: absorbs the Exp->Copy activation-table reload
        # while the PE is still on the first output chunk.
        warm = opool.tile([P, 8], f32, tag="warm")
        nc.scalar.activation(warm[:], cos_sb[:, 0:8], Copy)
        for nch in (0, 4, 1, 5, 2, 6, 3, 7):
            folded = nch < 4
            ot = opool.tile([P, N], bf16, tag="o")
            if folded:
                # folded chunks run through the psV ring (free in phase C):
                # no contention with the unfolded chunks' psD ring
                for cc in range(2):
                    ql = slice(cc * 512, (cc + 1) * 512)
                    pfh = psV.tile([P, 512], f32, tag="av")
                    for kc in range(2, 4):
                        nc.tensor.matmul(
                            pfh[:],
                            lhsT=aoT[:, kc, nch * P:(nch + 1) * P],
                            rhs=wo_sb[:, kc, cc * 512:(cc + 1) * 512],
                            start=(kc == 2),
                            stop=(kc == 3),
                        )
                    nc.vector.tensor_add(
                        out=ot[:, ql], in0=pfh[:], in1=o1[nch][:, ql]
                    )
                nc.sync.dma_start(out[nch * P:(nch + 1) * P, :], ot[:])
            else:
                # last chunk only: cc-outer so each 512-half's accumulation
                # group closes after its own 4 matmuls and the first
                # half-copy starts ~0.85us before the chunk's last matmul
                # (for earlier chunks kc-outer defers the norm(7)-gated kc=3)
                pf = psD.tile([P, N], f32, tag="mm")
                loops = ([(cc, kc) for cc in range(2) for kc in range(4)]
                         if nch == 7 else
                         [(cc, kc) for kc in range(4) for cc in range(2)])
                for cc, kc in loops:
                    nc.tensor.matmul(
                        pf[:, cc * 512:(cc + 1) * 512],
                        lhsT=aoT[:, kc, nch * P:(nch + 1) * P],
                        rhs=wo_sb[:, kc, cc * 512:(cc + 1) * 512],
                        start=(kc == 0),
                        stop=(kc == 3),
                    )
                if nch == 7:
                    # last chunk: per-half copies on the (now idle) Act
                    # engine with per-half stores — lowest drain latency
                    for q in range(2):
                        ql = slice(q * 512, (q + 1) * 512)
                        nc.scalar.activation(ot[:, ql], pf[:, ql], Copy)
                        nc.sync.dma_start(
                            out[nch * P:(nch + 1) * P, ql], ot[:, ql]
                        )
                else:
                    nc.scalar.activation(ot[:, 0:512], pf[:, 0:512], Copy)
                    nc.vector.tensor_copy(
                        out=ot[:, 512:1024], in_=pf[:, 512:1024]
                    )
                    nc.sync.dma_start(out[nch * P:(nch + 1) * P, :], ot[:])

    nc.compile()
    return nc


def _get_program():
    if "nc" not in _CACHE:
        _CACHE["nc"] = _build_program()
    return _CACHE["nc"]


def make_in_maps(x, context, rotary_pos, Wq, Wkv, Wout):
    from ml_dtypes import bfloat16

    x = np.asarray(x, dtype=np.float32)
    context = np.asarray(context, dtype=np.float32)
    rotary_pos = np.asarray(rotary_pos, dtype=np.float32)
    Wq = np.asarray(Wq, dtype=np.float32)
    Wkv = np.asarray(Wkv, dtype=np.float32)
    Wout = np.asarray(Wout, dtype=np.float32)

    def b16(a):
        return np.ascontiguousarray(a).astype(bfloat16)

    cosT = np.cos(rotary_pos).T  # [64, 1024]
    sinT = np.sin(rotary_pos).T
    sin_signed = np.concatenate([-sinT[:32], sinT[32:]], axis=0)
    cos2 = b16(np.vstack([cosT, cosT]))
    sin2 = b16(np.vstack([sin_signed, sin_signed]))

    in_maps = []
    for core in range(8):
        b, g = core // 2, core % 2
        cs = slice(g * ISH, (g + 1) * ISH)
        in_maps.append({
            "xbT": b16(x[b].T),
            "cxT": b16(context[b].T),
            "wq": b16(Wq[:, cs]),
            "wk": b16(Wkv[:, g * ISH:(g + 1) * ISH]),
            "wv": b16(Wkv[:, H * DH + g * ISH:H * DH + (g + 1) * ISH]),
            "wo": b16(Wout[cs, :]),
            "cos2": cos2,
            "sin2": sin2,
        })
    return in_maps


def kernel(x, context, mask, context_mask, rotary_pos, Wq, Wkv, Wout, bout):
    global _LAST_EXEC_NS
    from concourse.bass_utils import run_bass_kernel_spmd

    nc = _get_program()
    in_maps = make_in_maps(x, context, rotary_pos, Wq, Wkv, Wout)

    trace = bool(os.environ.get("BASS_KERNEL_TRACE"))
    res = run_bass_kernel_spmd(nc, in_maps, core_ids=list(range(8)), trace=trace)
    _LAST_EXEC_NS = res.exec_time_ns
    _CACHE["last_results"] = res

    bout = np.asarray(bout, dtype=np.float32)
    full = np.empty((B, N, DIM), dtype=np.float32)
    for b in range(B):
        full[b] = (
            res.results[2 * b]["out"].astype(np.float32)
            + res.results[2 * b + 1]["out"].astype(np.float32)
            + bout
        )
    return full


# revision 129
# speedup vs baseline: 1.0178x; 1.0026x over previous
"""CrossAttention Trainium2 kernel (bf16 pipeline).

Problem: nn_CrossAttention (B=4, N=M=1024, DIM=CTX_DIM=1024, H=16, DH=64).

Sharding: 8 cores = batch (4) x head-group (2 groups of 8 heads).
Each core computes, for its (b, g):
    q = rope(x[b] @ Wq[:, g])
    k = rope(context[b] @ Wk[:, g]);  v = context[b] @ Wv[:, g]
    attn = softmax(q k^T / sqrt(dh))     (mask is all-ones by construction)
    partial_out[b,g] = (attn @ v) @ Wout[g, :]
Host transposes x/context per batch and casts everything to bf16; it sums the
two head-group partials per batch and adds bout.

All matmuls run in bf16 (fp32 PSUM accumulation).  bf16 moving data streams at
1 cycle/row and the separate Ldweights instructions keep the PE p-state ramp
warm.  Dots/projection PSUM tiles are [128, 1024] (2 banks) so the Activation
engine amortizes its access latency over 1024-wide exp/cast chunks.

Engine assignment:
    PE    : all matmuls (projections, dots, attn@v, final)
    Act   : psum->bf16 casts feeding rope, exp(dots) -> es bf16, half the
            final copies
    DVE   : rope muls (bf16 SBUF, 2x perf mode), denominator reciprocal
            (reads PSUM rows 64-127 directly), normalize-mult fused with the
            psum->sbuf move of attn@v outputs, half the final copies
    Pool  : wq/cos/sin/wk loads via software DGE (fastest first-chunk
            latency; engine otherwise idle)
    SP    : x/context/wv/wo loads, half the rope-rotation DMAs, stores

Key tricks:
  - rope's rotate_half is a partition permutation (p XOR 32): done by small
    SBUF->SBUF DMAs (2 on SP, 2 on Act per chunk), because DVE tensor-tensor
    ops require equal SBUF start partitions (walrus
    checkSBSameStartPartition) while DMA addresses partitions freely.
  - v carries 64 ones-COLUMNS, so the attn@v matmul replicates the softmax
    denominator across PSUM rows 64-127 for free (matmul cost is moving-rows
    only); normalization is then reciprocal + one mul per half, all
    same-start.
  - head 0's dots+exp are interleaved with the v projection so the Act
    engine is warm when the (Act-exp-bound) attention loop starts; in the
    loop, attn_v(h) chunks are issued before the exp-ring-gated dots(h+1)
    chunks so the in-order PE queue never head-blocks.
  - the kc=0,1 half of the final projection for output chunks 0-3 runs
    inside attention iters 4-5 (psV ring slack, exp-paced PE slack); phase C
    then alternates folded chunks (kc 2,3 + one wide DVE add of the partial)
    with full chunks so the adds hide behind matmul time.
  - gpsimd partition_broadcast and rearrange-split-partition DMA APs both
    break on real hardware despite passing CoreSim/TimelineSim -- avoided.
"""

import os
import numpy as np

B, N, M = 4, 1024, 1024
DIM = 1024
H, DH = 16, 64
ISH = 512  # inner shard per core (8 heads * 64)
SCALE = DH ** -0.5
P = 128

_CACHE = {}
_LAST_EXEC_NS = None


def _build_program():
    from contextlib import ExitStack

    import concourse.tile as tile
    from concourse import bacc, mybir

    f32 = mybir.dt.float32
    bf16 = mybir.dt.bfloat16
    Exp = mybir.ActivationFunctionType.Exp
    Copy = mybir.ActivationFunctionType.Copy

    nc = bacc.Bacc("TRN2", target_bir_lowering=False, debug=False, num_devices=8)

    xbT = nc.dram_tensor("xbT", [DIM, N], bf16, kind="ExternalInput").ap()
    cxT = nc.dram_tensor("cxT", [DIM, M], bf16, kind="ExternalInput").ap()
    wq = nc.dram_tensor("wq", [DIM, ISH], bf16, kind="ExternalInput").ap()
    wk = nc.dram_tensor("wk", [DIM, ISH], bf16, kind="ExternalInput").ap()
    wv = nc.dram_tensor("wv", [DIM, ISH], bf16, kind="ExternalInput").ap()
    wo = nc.dram_tensor("wo", [ISH, DIM], bf16, kind="ExternalInput").ap()
    cos2 = nc.dram_tensor("cos2", [P, N], bf16, kind="ExternalInput").ap()
    sin2 = nc.dram_tensor("sin2", [P, N], bf16, kind="ExternalInput").ap()
    out = nc.dram_tensor("out", [N, DIM], bf16, kind="ExternalOutput").ap()

    with tile.TileContext(nc) as tc, ExitStack() as ctx:
        const = ctx.enter_context(tc.tile_pool(name="const", bufs=1))
        wpool = ctx.enter_context(tc.tile_pool(name="wpool", bufs=2))
        qk = ctx.enter_context(tc.tile_pool(name="qk", bufs=1))
        vpool = ctx.enter_context(tc.tile_pool(name="vpool", bufs=8))
        tmpp = ctx.enter_context(tc.tile_pool(name="tmpp", bufs=8))

        wq_sb = wpool.tile([P, 8, ISH], bf16, tag="w")
        wk_sb = wpool.tile([P, 8, ISH], bf16, tag="w")
        wv_sb = wpool.tile([P, 8, ISH], bf16, tag="w")
        for k in range(8):
            nc.gpsimd.dma_start(wq_sb[:, k, :], wq[k * P:(k + 1) * P, :])
        cos_sb = const.tile([P, N], bf16, tag="cos")
        nc.gpsimd.dma_start(cos_sb[:], cos2)
        sin_sb = const.tile([P, N], bf16, tag="sin")
        nc.gpsimd.dma_start(sin_sb[:], sin2)
        for k in range(8):
            nc.gpsimd.dma_start(wk_sb[:, k, :], wk[k * P:(k + 1) * P, :])

        # ---- phase A: projections (xT/ctxT big tiles live only here)
        psAB = ctx.enter_context(ExitStack())
        psD = psAB.enter_context(tc.tile_pool(name="psD", bufs=2, space="PSUM"))
        psV = psAB.enter_context(tc.tile_pool(name="psV", bufs=4, space="PSUM"))
        epool = ctx.enter_context(tc.tile_pool(name="epool", bufs=16))
        with tc.tile_pool(name="bigT", bufs=2) as bigT:

            def project_rope(xT, w_sb, tag):
                dst = qk.tile([P, 4, N], bf16, tag=tag)
                for ic in range(4):
                    ps = psD.tile([P, N], f32, tag="mm")
                    for k in range(8):
                        for ns in range(2):
                            lt = w_sb[:, k, ic * P:(ic + 1) * P]
                            if w_sb is wq_sb and ic == 0 and k == 0:
                                lt = wq00[:]
                            nc.tensor.matmul(
                                ps[:, ns * 512:(ns + 1) * 512],
                                lhsT=lt,
                                rhs=xT[:, k, ns * 512:(ns + 1) * 512],
                                start=(k == 0),
                                stop=(k == 7),
                            )
                    qc = tmpp.tile([P, N], bf16, tag="qc")
                    nc.scalar.activation(qc[:], ps[:], Copy)
                    # rope: dst = qc * cos + rotate_half(qc) * sin_signed.
                    # The partition rotation (p -> p XOR 32) runs on the DMA
                    # engines: DVE tensor-tensor ops require equal SBUF start
                    # partitions (walrus checkSBSameStartPartition), and DMA
                    # addresses partitions freely.  Issue split across the SP
                    # and DVE queues to fit their sequencer budgets.
                    qcr = tmpp.tile([P, N], bf16, tag="qcr")
                    for blk in range(4):
                        d0 = blk * 32
                        s0 = (blk ^ 1) * 32
                        eng = nc.sync
                        eng.dma_start(
                            qcr[d0:d0 + 32, :], qc[s0:s0 + 32, :]
                        )
                    dsl = dst[:, ic, :]
                    nc.vector.tensor_mul(out=dsl, in0=qc[:], in1=cos_sb[:])
                    tmp = tmpp.tile([P, N], bf16, tag="tmp")
                    nc.vector.tensor_mul(out=tmp[:], in0=qcr[:], in1=sin_sb[:])
                    nc.vector.tensor_add(out=dsl, in0=dsl, in1=tmp[:])
                return dst

            # DMA issue plan: SP carries wq0 (fastest path for the first
            # matmul) then x/context/wv; Act queue carries wq1-7 in parallel
            # and is free for the rope casts by ~5us; Pool carries cos/sin/wk
            # (software DGE, idle engine).
            wq00 = bigT.tile([P, P], bf16, tag="wq00")
            nc.scalar.dma_start(wq00[:], wq[0:P, 0:P])
            xT = bigT.tile([P, 8, N], bf16, tag="bigT")
            for k in range(8):
                nc.sync.dma_start(xT[:, k, :], xbT[k * P:(k + 1) * P, :])
            cT = bigT.tile([P, 8, N], bf16, tag="bigT")
            for k in range(8):
                nc.sync.dma_start(cT[:, k, :], cxT[k * P:(k + 1) * P, :])
            for k in range(8):
                nc.sync.dma_start(wv_sb[:, k, :], wv[k * P:(k + 1) * P, :])
            qT = project_rope(xT, wq_sb, "qT")
            kT = project_rope(cT, wk_sb, "kT")

            def dots_exp0_mch(mch, es):
                # head 0's dots+exp through the (phase-A-idle) attention psum
                # ring, interleaved with the v projection so the Act engine
                # stays busy through phase A's tail
                e = epool.tile([P, N], bf16, tag="e")
                for ns in range(2):
                    psd = psV.tile([P, 512], f32, tag="av")
                    nc.tensor.matmul(
                        psd[:],
                        lhsT=kT[0:64, 0, mch * P:(mch + 1) * P],
                        rhs=qT[0:64, 0, ns * 512:(ns + 1) * 512],
                        start=True,
                        stop=True,
                    )
                    nc.scalar.activation(
                        e[:, ns * 512:(ns + 1) * 512], psd[:], Exp,
                        scale=SCALE,
                    )
                es.append(e)

            # vt tiles allocated upfront: the ones-columns memsets run at
            # program start on the idle DVE instead of inside the v-window
            vsb = [vpool.tile([P, 8, 2 * DH], bf16, tag="v", name=f"vt{_i}")
                   for _i in range(8)]
            for vt in vsb:
                nc.vector.memset(vt[:, :, DH:2 * DH], 1.0)
            es0 = []
            for mp in range(4):
                ps = psD.tile([P, N], f32, tag="mm")
                for half in range(2):
                    mch = mp * 2 + half
                    for k in range(8):
                        nc.tensor.matmul(
                            ps[:, half * 512:(half + 1) * 512],
                            lhsT=cT[:, k, mch * P:(mch + 1) * P],
                            rhs=wv_sb[:, k, :],
                            start=(k == 0),
                            stop=(k == 7),
                        )
                for half in range(2):
                    # 64 ones-columns (memset upfront): the attn@v matmul
                    # replicates the softmax denominator across PSUM rows
                    # 64-127, so the partition broadcast of 1/denom is free.
                    # Copy on DVE: keeps the Act queue clear for the es0
                    # exps that pace the dots0 psum ring.
                    vt = vsb[mp * 2 + half]
                    nc.vector.tensor_copy(
                        out=vt[:, :, 0:DH],
                        in_=ps[:, half * 512:(half + 1) * 512].rearrange(
                            "p (h d) -> p h d", d=DH
                        ),
                    )
                dots_exp0_mch(2 * mp, es0)
                dots_exp0_mch(2 * mp + 1, es0)

        # ---- phase B: attention (bigT space now free)
        rcp = ctx.enter_context(tc.tile_pool(name="rcp", bufs=4))
        rbp = ctx.enter_context(tc.tile_pool(name="rbp", bufs=4))
        drp = ctx.enter_context(tc.tile_pool(name="drp", bufs=4, space="DRAM"))
        opool = ctx.enter_context(tc.tile_pool(name="opool", bufs=6))

        aoT = qk.tile([P, 4, N], bf16, tag="aoT")

        wo_sb = wpool.tile([P, 4, DIM], bf16, tag="w")
        for k in range(4):
            nc.sync.dma_start(wo_sb[:, k, :], wo[k * P:(k + 1) * P, :])

        def denom_normalize(h, pos):
            # PSUM rows 64-127 already hold the denominator replicated (ones
            # columns of v): move to sbuf, reciprocal, normalize.  All SBUF
            # operand pairs share start partitions.
            t2, r0 = h // 2, (h % 2) * 64
            rb = rbp.tile([P, N], f32, tag="rb")
            for ns in range(2):
                nsl = slice(ns * 512, (ns + 1) * 512)
                with nc.allow_low_precision(reason="softmax denom recip"):
                    nc.vector.reciprocal(
                        out=rb[r0:r0 + 64, nsl], in_=pos[ns][DH:2 * DH, :]
                    )
                nc.vector.tensor_mul(
                    out=aoT[r0:r0 + 64, t2, nsl],
                    in0=pos[ns][0:DH, :],
                    in1=rb[r0:r0 + 64, nsl],
                )

        # Main attention loop.  dots(h+1) and attn_v(h) are interleaved at
        # chunk granularity: the dots matmuls are gated by the exp-paced psD
        # ring, and the in-order PE queue would otherwise head-block the
        # (dependency-free) attn_v matmuls behind them.
        o1pool = ctx.enter_context(tc.tile_pool(name="o1pool", bufs=4))
        o1 = []
        es_cur = es0
        for h in range(8):
            if h < 7:
                t2, r0 = (h + 1) // 2, ((h + 1) % 2) * 64
                qh = qT[r0:r0 + 64, t2, :]
                kh = kT[r0:r0 + 64, t2, :]
            es_next = []
            pos = [psV.tile([P, 512], f32, tag="av", name=f"po{_i}")
                   for _i in range(2)]
            for mch in range(8):
                for ns in range(2):
                    nc.tensor.matmul(
                        pos[ns][:],
                        lhsT=vsb[mch][:, h, :],
                        rhs=es_cur[mch][:, ns * 512:(ns + 1) * 512],
                        start=(mch == 0),
                        stop=(mch == 7),
                    )
                if h < 7:
                    psd = psD.tile([P, N], f32, tag="mm")
                    for ns in range(2):
                        nc.tensor.matmul(
                            psd[:, ns * 512:(ns + 1) * 512],
                            lhsT=kh[:, mch * P:(mch + 1) * P],
                            rhs=qh[:, ns * 512:(ns + 1) * 512],
                            start=True,
                            stop=True,
                        )
                    e = epool.tile([P, N], bf16, tag="e")
                    nc.scalar.activation(e[:], psd[:], Exp, scale=SCALE)
                    es_next.append(e)
            denom_normalize(h, pos)
            es_cur = es_next
            # fold the kc=0,1 half of the final projection for nch 0-3 into
            # iters 4-5: psV has two spare ring slots there and the iters are
            # exp-paced with ~1.5us of PE slack
            if h in (4, 5):
                for nch in (2 * (h - 4), 2 * (h - 4) + 1):
                    o1t = o1pool.tile([P, N], f32, tag="o1",
                                      name=f"o1_{nch}")
                    for cc in range(2):
                        pf1 = psV.tile([P, 512], f32, tag="av")
                        for kc in range(2):
                            nc.tensor.matmul(
                                pf1[:],
                                lhsT=aoT[:, kc, nch * P:(nch + 1) * P],
                                rhs=wo_sb[:, kc, cc * 512:(cc + 1) * 512],
                                start=(kc == 0),
                                stop=(kc == 1),
                            )
                        nc.vector.tensor_copy(
                            out=o1t[:, cc * 512:(cc + 1) * 512], in_=pf1[:]
                        )
                    o1.append(o1t)

        # ---- final projection.  psD ring is free immediately (unlike psV,
        # whose last slots wait on norm(7)); only the kc=3 matmuls depend on
        # the last head's normalize chain.  Folded chunks (kc 2,3 + add of
        # the phase-B partial) alternate with full chunks so the DVE adds
        # hide behind the full chunks' matmul time.
        # Tiny Copy first: absorbs the Exp->Copy activation-table reload
        # while the PE is still on the first output chunk.
        warm = opool.tile([P, 8], f32, tag="warm")
        nc.scalar.activation(warm[:], cos_sb[:, 0:8], Copy)
        for nch in (0, 4, 1, 5, 2, 6, 3, 7):
            folded = nch < 4
            ot = opool.tile([P, N], bf16, tag="o")
            if folded:
                # folded chunks run through the psV ring (free in phase C):
                # no contention with the unfolded chunks' psD ring
                for cc in range(2):
                    ql = slice(cc * 512, (cc + 1) * 512)
                    pfh = psV.tile([P, 512], f32, tag="av")
                    for kc in range(2, 4):
                        nc.tensor.matmul(
                            pfh[:],
                            lhsT=aoT[:, kc, nch * P:(nch + 1) * P],
                            rhs=wo_sb[:, kc, cc * 512:(cc + 1) * 512],
                            start=(kc == 2),
                            stop=(kc == 3),
                        )
                    nc.vector.tensor_add(
                        out=ot[:, ql], in0=pfh[:], in1=o1[nch][:, ql]
                    )
                nc.sync.dma_start(out[nch * P:(nch + 1) * P, :], ot[:])
            else:
                # last chunk only: cc-outer so each 512-half's accumulation
                # group closes after its own 4 matmuls and the first
                # half-copy starts ~0.85us before the chunk's last matmul
                # (for earlier chunks kc-outer defers the norm(7)-gated kc=3)
                pf = psD.tile([P, N], f32, tag="mm")
                loops = ([(cc, kc) for cc in range(2) for kc in range(4)]
                         if nch == 7 else
                         [(cc, kc) for kc in range(4) for cc in range(2)])
                for cc, kc in loops:
                    nc.tensor.matmul(
                        pf[:, cc * 512:(cc + 1) * 512],
                        lhsT=aoT[:, kc, nch * P:(nch + 1) * P],
                        rhs=wo_sb[:, kc, cc * 512:(cc + 1) * 512],
                        start=(kc == 0),
                        stop=(kc == 3),
                    )
                if nch == 7:
                    # last chunk: per-half copies on the (now idle) Act
                    # engine with per-half stores — lowest drain latency
                    for q in range(2):
                        ql = slice(q * 512, (q + 1) * 512)
                        nc.scalar.activation(ot[:, ql], pf[:, ql], Copy)
                        nc.sync.dma_start(
                            out[nch * P:(nch + 1) * P, ql], ot[:, ql]
                        )
                else:
                    nc.scalar.activation(ot[:, 0:512], pf[:, 0:512], Copy)
                    nc.vector.tensor_copy(
                        out=ot[:, 512:1024], in_=pf[:, 512:1024]
                    )
                    nc.sync.dma_start(out[nch * P:(nch + 1) * P, :], ot[:])

    nc.compile()
    return nc


def _get_program():
    if "nc" not in _CACHE:
        _CACHE["nc"] = _build_program()
    return _CACHE["nc"]


def make_in_maps(x, context, rotary_pos, Wq, Wkv, Wout):
    from ml_dtypes import bfloat16

    x = np.asarray(x, dtype=np.float32)
    context = np.asarray(context, dtype=np.float32)
    rotary_pos = np.asarray(rotary_pos, dtype=np.float32)
    Wq = np.asarray(Wq, dtype=np.float32)
    Wkv = np.asarray(Wkv, dtype=np.float32)
    Wout = np.asarray(Wout, dtype=np.float32)

    def b16(a):
        return np.ascontiguousarray(a).astype(bfloat16)

    cosT = np.cos(rotary_pos).T  # [64, 1024]
    sinT = np.sin(rotary_pos).T
    sin_signed = np.concatenate([-sinT[:32], sinT[32:]], axis=0)
    cos2 = b16(np.vstack([cosT, cosT]))
    sin2 = b16(np.vstack([sin_signed, sin_signed]))

    in_maps = []
    for core in range(8):
        b, g = core // 2, core % 2
        cs = slice(g * ISH, (g + 1) * ISH)
        in_maps.append({
            "xbT": b16(x[b].T),
            "cxT": b16(context[b].T),
            "wq": b16(Wq[:, cs]),
            "wk": b16(Wkv[:, g * ISH:(g + 1) * ISH]),
            "wv": b16(Wkv[:, H * DH + g * ISH:H * DH + (g + 1) * ISH]),
            "wo": b16(Wout[cs, :]),
            "cos2": cos2,
            "sin2": sin2,
        })
    return in_maps


def kernel(x, context, mask, context_mask, rotary_pos, Wq, Wkv, Wout, bout):
    global _LAST_EXEC_NS
    from concourse.bass_utils import run_bass_kernel_spmd

    nc = _get_program()
    in_maps = make_in_maps(x, context, rotary_pos, Wq, Wkv, Wout)

    trace = bool(os.environ.get("BASS_KERNEL_TRACE"))
    res = run_bass_kernel_spmd(nc, in_maps, core_ids=list(range(8)), trace=trace)
    _LAST_EXEC_NS = res.exec_time_ns
    _CACHE["last_results"] = res

    bout = np.asarray(bout, dtype=np.float32)
    full = np.empty((B, N, DIM), dtype=np.float32)
    for b in range(B):
        full[b] = (
            res.results[2 * b]["out"].astype(np.float32)
            + res.results[2 * b + 1]["out"].astype(np.float32)
            + bout
        )
    return full
